# revision 2
# baseline (speedup 1.0000x reference)
"""GPT forward pass on 8 Trainium2 NeuronCores (Bass/Tile), bf16 compute.

Model: B=2, S=1024, D=1024, H=16 heads (hd=64), L=6 layers, V=50257,
tied embedding head.

Sharding: DP2 x TP4. Cores 0-3 compute batch element 0, cores 4-7
batch element 1. Within each group of 4: attention is head-sharded
(4 heads/core), the FFN hidden dim is column/row sharded (1024/core),
and the tied logit matrix is vocab-sharded (12565 rows/core, padded
to 12800). Two AllReduces per layer (post-attention, post-FFN) over
each group of 4, token-chunked (2 x 512) so collectives overlap
compute.

On-device layout: activations are feature-major ("transposed"):
x[p, k, t] = X^T[128k + p, t]. All matmuls take weights [in, out] as
the stationary operand and activations [in, tokens] as the moving
operand, producing outputs feature-major with zero activation
transposes. LayerNorm gains/biases and the attention 1/sqrt(hd) scale
are folded into the adjacent weights on the host, so the device only
computes the pure normalization (x - mean) * rsqrt(var + eps), with
stats via ones-matmuls on the PE and per-token broadcasts via GpSimd.

All matmul datapaths are bf16 (weights cast on host, activations cast
on write); PSUM accumulation stays fp32, the residual stream stays
fp32 in SBUF, and the AllReduce payload is bf16.
"""

import sys

sys.path.insert(0, "/opt/trn_rl_repo")

import contextlib

import numpy as np

import concourse.bacc as bacc
import concourse.mybir as mybir
import concourse.tile as tile
from concourse.bass import ts
from concourse.bass_utils import run_bass_kernel_spmd

F32 = mybir.dt.float32
BF16 = mybir.dt.bfloat16
AF = mybir.ActivationFunctionType
ALU = mybir.AluOpType

# Model dims
B, S, D, H, L, V = 2, 1024, 1024, 16, 6, 50257
HD = D // H           # 64
DFF = 4 * D           # 4096
N_CORES = 8
TP = 4                # tensor-parallel group size
HPC = H // TP         # heads per core = 4
DQ = HPC * HD         # per-core qkv width = 256
FFC = DFF // TP       # per-core ffn hidden = 1024
KD = D // 128         # 8 feature tiles
T = S                 # tokens per core (one batch element)
TCH = 512             # token chunk for AR pipelining
NTC = T // TCH        # 2
VS = 12565            # vocab rows per core (last core: 12562)
VSP = 12800           # padded vocab rows per core
MASK_VAL = -60.0

REPLICA_GROUPS = [[0, 1, 2, 3], [4, 5, 6, 7]]


def _f(name, l=None):
    return name if l is None else f"{name}{l}"


def build_program():
    """Build the SPMD bass program (same instruction stream on all cores)."""
    nc = bacc.Bacc("TRN2", target_bir_lowering=False, debug=False,
                   enable_asserts=True, num_devices=N_CORES)

    inp = {}

    def dram_in(name, shape, dtype=BF16):
        inp[name] = nc.dram_tensor(name, shape, dtype, kind="ExternalInput").ap()
        return inp[name]

    dram_in("x0", [128, KD, T], F32)
    dram_in("ones", [128, 1])
    dram_in("ident", [128, 128])
    dram_in("ident2", [128, 64])
    dram_in("tri", [128, 128])
    for l in range(L):
        dram_in(_f("wq", l), [128, KD, DQ])
        dram_in(_f("wk", l), [128, KD, DQ])
        dram_in(_f("wv", l), [128, KD, DQ])
        dram_in(_f("wo", l), [128, DQ // 128, D])
        dram_in(_f("w1", l), [128, KD, FFC])
        dram_in(_f("w2", l), [128, FFC // 128, D])
        dram_in(_f("bqkv", l), [128, 6], F32)
        dram_in(_f("b1", l), [128, FFC // 128], F32)
        dram_in(_f("b2", l), [128, KD], F32)
    dram_in("emb", [128, KD, VSP])
    logits = nc.dram_tensor("logits", [VSP, T], F32, kind="ExternalOutput").ap()

    with tile.TileContext(nc) as tc:
        _body(tc, inp, logits)
    nc.compile()
    return nc


def _body(tc, inp, logits):
    nc = tc.nc
    ctx = contextlib.ExitStack()
    with ctx:
        # --- SBUF pools (sizes are KB/partition) ---
        singles = ctx.enter_context(tc.tile_pool(name="singles", bufs=1))
        xp = ctx.enter_context(tc.tile_pool(name="xp", bufs=1))        # 32
        npool = ctx.enter_context(tc.tile_pool(name="npool", bufs=1))  # 16
        xbp = ctx.enter_context(tc.tile_pool(name="xbp", bufs=2))      # 16
        tmp = ctx.enter_context(tc.tile_pool(name="tmp", bufs=2))      # 5
        qkv = ctx.enter_context(tc.tile_pool(name="qkv", bufs=1))      # 12
        vh = ctx.enter_context(tc.tile_pool(name="vh", bufs=4))        # 2
        ep = ctx.enter_context(tc.tile_pool(name="ep", bufs=2))        # 4
        et = ctx.enter_context(tc.tile_pool(name="et", bufs=1))        # 8
        op = ctx.enter_context(tc.tile_pool(name="op", bufs=1))        # 4
        hp = ctx.enter_context(tc.tile_pool(name="hp", bufs=1))        # 8
        arr = ctx.enter_context(tc.tile_pool(name="arr", bufs=1))      # 8
        wts = ctx.enter_context(tc.tile_pool(name="wts", bufs=2))      # 6
        embp = ctx.enter_context(tc.tile_pool(name="embp", bufs=2))    # 8
        stat = ctx.enter_context(tc.tile_pool(name="stat", bufs=2))
        bcp = ctx.enter_context(tc.tile_pool(name="bcp", bufs=1))      # 8
        lout = ctx.enter_context(tc.tile_pool(name="lout", bufs=1))
        bias = ctx.enter_context(tc.tile_pool(name="bias", bufs=3))
        stg = ctx.enter_context(tc.tile_pool(name="stg", bufs=2))      # 3
        # --- PSUM pools (8 banks total) ---
        ps_mm = ctx.enter_context(tc.tile_pool(name="ps_mm", bufs=2, space="PSUM"))
        ps_st = ctx.enter_context(tc.tile_pool(name="ps_st", bufs=1, space="PSUM"))
        ps_sc = ctx.enter_context(tc.tile_pool(name="ps_sc", bufs=2, space="PSUM"))
        ps_tr = ctx.enter_context(tc.tile_pool(name="ps_tr", bufs=2, space="PSUM"))
        ps_o = ctx.enter_context(tc.tile_pool(name="ps_o", bufs=1, space="PSUM"))
        # --- DRAM (collective bounce) ---
        dram = ctx.enter_context(tc.tile_pool(name="dram", bufs=4, space="DRAM"))

        # --- constants / persistent ---
        ones_t = singles.tile([128, 1], BF16)
        nc.sync.dma_start(out=ones_t[:], in_=inp["ones"][:])
        ident_t = singles.tile([128, 128], BF16)
        nc.sync.dma_start(out=ident_t[:], in_=inp["ident"][:])
        ident2_t = singles.tile([128, 64], BF16)
        nc.sync.dma_start(out=ident2_t[:], in_=inp["ident2"][:])
        tri_t = singles.tile([128, 128], BF16)
        nc.sync.dma_start(out=tri_t[:], in_=inp["tri"][:])
        eps_t = singles.tile([1, 1], F32)
        nc.vector.memset(eps_t[:], 1e-5)

        xt = xp.tile([128, KD, T], F32, tag="x")
        nc.sync.dma_start(out=xt[:], in_=inp["x0"][:])

        def layer_norm_chunk(src, dst, c):
            """dst[:,:,c] (bf16) = (src - mean) * rsqrt(var + eps)."""
            cs = ts(c, TCH)
            xb = xbp.tile([128, KD, TCH], BF16, tag="xb")
            for k in range(KD):
                nc.gpsimd.tensor_scalar_add(xb[:, k, :], src[:, k, cs], 0.0)
            s1 = ps_st.tile([1, TCH], F32, tag="st")
            s2 = ps_st.tile([1, TCH], F32, tag="st")
            for k in range(KD):
                nc.tensor.matmul(s1[:], ones_t[:], xb[:, k, :],
                                 start=(k == 0), stop=(k == KD - 1))
            for k in range(KD):
                sq = tmp.tile([128, TCH], BF16, tag="tmp")
                nc.vector.tensor_tensor(
                    out=sq[:], in0=xb[:, k, :], in1=xb[:, k, :], op=ALU.mult)
                nc.tensor.matmul(s2[:], ones_t[:], sq[:],
                                 start=(k == 0), stop=(k == KD - 1))
            # finishing math on [1, TCH] rows
            m = stat.tile([1, TCH], F32, tag="sa")
            nc.vector.tensor_scalar_mul(m[:], s1[:], 1.0 / D)
            msq = stat.tile([1, TCH], F32, tag="sb")
            nc.vector.tensor_tensor(out=msq[:], in0=m[:], in1=m[:],
                                    op=ALU.mult)
            var = stat.tile([1, TCH], F32, tag="sb")
            nc.vector.scalar_tensor_tensor(
                out=var[:], in0=s2[:], scalar=1.0 / D, in1=msq[:],
                op0=ALU.mult, op1=ALU.subtract)
            rs = stat.tile([1, TCH], F32, tag="sb")
            nc.scalar.activation(rs[:], var[:], AF.Sqrt, bias=eps_t[:])
            nc.vector.reciprocal(rs[:], rs[:])
            a = stat.tile([1, TCH], F32, tag="sa")
            nc.vector.scalar_tensor_tensor(
                out=a[:], in0=m[:], scalar=-1.0, in1=rs[:],
                op0=ALU.mult, op1=ALU.mult)
            rB = bcp.tile([128, TCH], F32, tag="rB")
            nc.gpsimd.partition_broadcast(rB[:], rs[:])
            aB = bcp.tile([128, TCH], F32, tag="aB")
            nc.gpsimd.partition_broadcast(aB[:], a[:])
            # dst = src * rB + aB   (two DVE passes per k)
            for k in range(KD):
                t2 = tmp.tile([128, TCH], F32, tag="tmp2")
                nc.vector.tensor_tensor(
                    out=t2[:], in0=src[:, k, cs], in1=rB[:], op=ALU.mult)
                nc.vector.tensor_tensor(
                    out=dst[:, k, cs], in0=t2[:], in1=aB[:], op=ALU.add)

        def proj_chunk(dst, wname, n_src, mchunks, bias_t, bcol0, c):
            """dst[:, m, c-chunk] (bf16) = W^T @ n_src[c-chunk] + bias."""
            cs = ts(c, TCH)
            for m in range(mchunks):
                wstrip = wts.tile([128, KD, 128], BF16, tag="w")
                nc.sync.dma_start(out=wstrip[:],
                                  in_=inp[wname][:, :, ts(m, 128)])
                pt = ps_mm.tile([128, TCH], F32, tag="mm")
                for k in range(KD):
                    nc.tensor.matmul(pt[:], wstrip[:, k, :],
                                     n_src[:, k, cs],
                                     start=(k == 0), stop=(k == KD - 1))
                nc.scalar.activation(
                    dst[:, m, cs], pt[:], AF.Identity,
                    bias=bias_t[:, bcol0 + m:bcol0 + m + 1])

        def qkv_head_chunk(l, c, n_src, qT, kT, vT, bqkv_t, vhs):
            """QKV projections for token chunk c, plus the v-transposes
            whose key blocks live in chunk c."""
            proj_chunk(qT, _f("wq", l), n_src, DQ // 128, bqkv_t, 0, c)
            proj_chunk(kT, _f("wk", l), n_src, DQ // 128, bqkv_t, 2, c)
            proj_chunk(vT, _f("wv", l), n_src, DQ // 128, bqkv_t, 4, c)
            for h in range(HPC):
                pp = 64 * (h % 2)
                mh = h // 2
                for j in range(4 * c, 4 * c + 4):
                    tp_ = ps_tr.tile([128, 128], BF16, tag="tr")
                    nc.tensor.matmul(tp_[:, 0:HD],
                                     vT[pp:pp + 64, mh, ts(j, 128)],
                                     ident2_t[pp:pp + 64, :],
                                     is_transpose=True,
                                     start=True, stop=True)
                    nc.scalar.copy(vhs[h][:, j, :], tp_[:, 0:HD])

        def attn_chunk(l, c, qT, kT, vhs, oT, inject=None, inject_at=1):
            for h in range(HPC):
                if h == inject_at and inject is not None:
                    inject()
                pp = 64 * (h % 2)
                mh = h // 2
                etile = et.tile([128, KD, TCH], BF16, tag="et")
                nkj = 4 * (c + 1)
                for qi in range(4 * c, 4 * c + 4):
                    qs = ts(qi, 128)
                    nkeys = 128 * (qi + 1)
                    erow = ep.tile([128, T], BF16, tag="e")
                    rsum = stat.tile([128, 1], F32, tag="rsum")
                    nchunks = (nkeys + 511) // 512
                    for sc in range(nchunks):
                        w = min(512, nkeys - 512 * sc)
                        last = sc == nchunks - 1
                        spt = ps_sc.tile([128, 512], F32, tag="sc")
                        nc.tensor.matmul(
                            spt[:, :w], qT[pp:pp + 64, mh, qs],
                            kT[pp:pp + 64, mh, 512 * sc:512 * sc + w],
                            start=True, stop=not last)
                        if last:
                            # add the causal mask for the diagonal block
                            nc.tensor.matmul(spt[:, w - 128:w], ident_t[:],
                                             tri_t[:], start=False, stop=True)
                        if sc == 0:
                            nc.scalar.activation(
                                erow[:, :w], spt[:, :w], AF.Exp,
                                accum_out=rsum[:])
                        else:
                            rpart = stat.tile([128, 1], F32, tag="rp")
                            nc.scalar.activation(
                                erow[:, 512 * sc:512 * sc + w],
                                spt[:, :w], AF.Exp, accum_out=rpart[:])
                            nc.vector.tensor_tensor(
                                out=rsum[:], in0=rsum[:], in1=rpart[:],
                                op=ALU.add)
                    nc.vector.reciprocal(rsum[:], rsum[:])
                    en = ep.tile([128, T], BF16, tag="e")
                    nc.scalar.activation(en[:, :nkeys], erow[:, :nkeys],
                                         AF.Copy, scale=rsum[:])
                    for kj in range(qi + 1):
                        tp_ = ps_tr.tile([128, 128], BF16, tag="tr")
                        nc.tensor.matmul(tp_[:], en[:, ts(kj, 128)],
                                         ident_t[:], is_transpose=True,
                                         start=True, stop=True)
                        qo = 128 * (qi - 4 * c)
                        nc.scalar.copy(etile[:, kj, qo:qo + 128], tp_[:])
                po = ps_o.tile([64, TCH], F32, tag="o")
                for kj in range(nkj):
                    lo = max(0, 128 * kj - TCH * c)
                    nc.tensor.matmul(po[:, lo:TCH], vhs[h][:, kj, :],
                                     etile[:, kj, lo:TCH],
                                     start=(kj == 0), stop=(kj == nkj - 1))
                nc.scalar.copy(oT[pp:pp + 64, mh, ts(c, TCH)], po[:])

        def mm_ar_chunk(wname, kchunks, src_tile, c, src_is_chunk):
            """out-partial = W^T @ src for chunk c -> bf16 AllReduce."""
            ar_in = dram.tile([128, KD, TCH], BF16, tag="dr")
            ar_out = dram.tile([128, KD, TCH], BF16, tag="dr")
            for m in range(KD):
                wstrip = wts.tile([128, KD, 128], BF16, tag="w")
                nc.sync.dma_start(out=wstrip[:, 0:kchunks, :],
                                  in_=inp[wname][:, :, ts(m, 128)])
                pt = ps_mm.tile([128, TCH], F32, tag="mm")
                for k in range(kchunks):
                    s = (src_tile[:, k, :] if src_is_chunk
                         else src_tile[:, k, ts(c, TCH)])
                    nc.tensor.matmul(pt[:], wstrip[:, k, :], s,
                                     start=(k == 0), stop=(k == kchunks - 1))
                st_ = stg.tile([128, TCH], BF16, tag="stg")
                nc.scalar.copy(st_[:], pt[:])
                nc.sync.dma_start(out=ar_in[:, m, :], in_=st_[:])
            nc.gpsimd.collective_compute(
                "AllReduce", ALU.add, replica_groups=REPLICA_GROUPS,
                ins=[ar_in.opt()], outs=[ar_out.opt()])
            return ar_out

        # ---------------- prologue: LN1 + QKV of layer 0 ----------------
        n_cur = npool.tile([128, KD, T], BF16, tag="n")
        bqkv_t = bias.tile([128, 6], F32, tag="bias")
        nc.sync.dma_start(out=bqkv_t[:], in_=inp[_f("bqkv", 0)][:])
        qT = qkv.tile([128, DQ // 128, T], BF16, tag="qT")
        kT = qkv.tile([128, DQ // 128, T], BF16, tag="kT")
        vT = qkv.tile([128, DQ // 128, T], BF16, tag="vT")
        vhs = [vh.tile([128, KD, HD], BF16, tag="vh", name=f"vh{i}")
               for i in range(HPC)]
        for c in range(NTC):
            layer_norm_chunk(xt, n_cur, c)
            qkv_head_chunk(0, c, n_cur, qT, kT, vT, bqkv_t, vhs)

        def ffn_w1(l, c, n2, b1_t):
            cs = ts(c, TCH)
            hT = hp.tile([128, FFC // 128, TCH], BF16, tag="h",
                         name=f"hT{l}_{c}")
            for m in range(FFC // 128):
                wstrip = wts.tile([128, KD, 128], BF16, tag="w",
                                  name=f"w1s{l}_{c}_{m}")
                nc.sync.dma_start(out=wstrip[:],
                                  in_=inp[_f("w1", l)][:, :, ts(m, 128)])
                pt = ps_mm.tile([128, TCH], F32, tag="mm", name=f"p1_{l}_{c}_{m}")
                for k in range(KD):
                    nc.tensor.matmul(pt[:], wstrip[:, k, :], n2[:, k, cs],
                                     start=(k == 0), stop=(k == KD - 1))
                nc.scalar.activation(hT[:, m, :], pt[:], AF.Gelu,
                                     bias=b1_t[:, m:m + 1])
            return hT

        def add_f_lnnext(c, ar_f, b2_t, n_next):
            cs = ts(c, TCH)
            art = arr.tile([128, KD, TCH], BF16, tag="arr", name=f"artf{c}")
            nc.sync.dma_start(out=art[:], in_=ar_f[c][:])
            for m in range(KD):
                nc.vector.scalar_tensor_tensor(
                    out=xt[:, m, cs], in0=art[:, m, :],
                    scalar=b2_t[:, m:m + 1],
                    in1=xt[:, m, cs],
                    op0=ALU.add, op1=ALU.add)
            layer_norm_chunk(xt, n_next, c)

        for l in range(L):
            # ---------------- attention + Wo + AR, chunk-pipelined ----------------
            oT = op.tile([128, DQ // 128, T], BF16, tag="oT")
            n2 = npool.tile([128, KD, T], BF16, tag="n")
            ar_a = []

            def add_a(c):
                art = arr.tile([128, KD, TCH], BF16, tag="arr",
                               name=f"arta{c}_{l}")
                nc.sync.dma_start(out=art[:], in_=ar_a[c][:])
                nc.vector.tensor_tensor(
                    out=xt[:, :, ts(c, TCH)],
                    in0=xt[:, :, ts(c, TCH)],
                    in1=art[:], op=ALU.add)

            def add_a_ln2_c0():
                add_a(0)
                layer_norm_chunk(xt, n2, 0)

            attn_chunk(l, 0, qT, kT, vhs, oT)
            ar_a.append(mm_ar_chunk(_f("wo", l), DQ // 128, oT, 0, False))
            attn_chunk(l, 1, qT, kT, vhs, oT, inject=add_a_ln2_c0)
            ar_a.append(mm_ar_chunk(_f("wo", l), DQ // 128, oT, 1, False))

            # ------- residual c1 + LN2(c1) overlap FFN(c0) -------
            b1_t = bias.tile([128, FFC // 128], F32, tag="bias")
            nc.sync.dma_start(out=b1_t[:], in_=inp[_f("b1", l)][:])
            b2_t = bias.tile([128, KD], F32, tag="bias")
            nc.sync.dma_start(out=b2_t[:], in_=inp[_f("b2", l)][:])
            add_a(1)
            layer_norm_chunk(xt, n2, 1)

            ar_f = []
            hT0 = ffn_w1(l, 0, n2, b1_t)
            ar_f.append(mm_ar_chunk(_f("w2", l), FFC // 128, hT0, 0, True))
            hT1 = ffn_w1(l, 1, n2, b1_t)

            n_next = npool.tile([128, KD, T], BF16, tag="n")
            ar_f.append(mm_ar_chunk(_f("w2", l), FFC // 128, hT1, 1, True))
            add_f_lnnext(0, ar_f, b2_t, n_next)

            if l < L - 1:
                bqkv_t = bias.tile([128, 6], F32, tag="bias")
                nc.sync.dma_start(out=bqkv_t[:], in_=inp[_f("bqkv", l + 1)][:])
                qT = qkv.tile([128, DQ // 128, T], BF16, tag="qT")
                kT = qkv.tile([128, DQ // 128, T], BF16, tag="kT")
                vT = qkv.tile([128, DQ // 128, T], BF16, tag="vT")
                vhs = [vh.tile([128, KD, HD], BF16, tag="vh",
                               name=f"vh{l + 1}_{i}") for i in range(HPC)]
                qkv_head_chunk(l + 1, 0, n_next, qT, kT, vT, bqkv_t, vhs)
            add_f_lnnext(1, ar_f, b2_t, n_next)
            if l < L - 1:
                qkv_head_chunk(l + 1, 1, n_next, qT, kT, vT, bqkv_t, vhs)
            n_cur = n_next

        # after the loop, n_cur holds the final layer norm
        nf = n_cur

        # logits^T [VSP, T]: emb strip is the stationary operand, nf moves.
        # A t2=0-only prefix runs first: it depends only on chunk 0 of the
        # final layernorm, hiding the last AllReduce + LN tail of chunk 1.
        NPRE = 16

        def logit_block(vb, t2, ebt):
            pt = ps_mm.tile([128, TCH], F32, tag="mm",
                            name=f"plg{vb}_{t2}")
            for k in range(KD):
                nc.tensor.matmul(pt[:], ebt[:, k, :],
                                 nf[:, k, ts(t2, TCH)],
                                 start=(k == 0), stop=(k == KD - 1))
            lo = lout.tile([128, TCH], F32, tag="lo", name=f"lo{vb}_{t2}")
            nc.scalar.copy(lo[:], pt[:])
            nc.sync.dma_start(out=logits[ts(vb, 128), ts(t2, TCH)],
                              in_=lo[:])

        for vb in range(NPRE):
            ebt = embp.tile([128, KD, 128], BF16, tag="emb", name=f"ebA{vb}")
            nc.sync.dma_start(out=ebt[:], in_=inp["emb"][:, :, ts(vb, 128)])
            logit_block(vb, 0, ebt)
        for vb in range(VSP // 128):
            ebt = embp.tile([128, KD, 128], BF16, tag="emb", name=f"ebB{vb}")
            nc.sync.dma_start(out=ebt[:], in_=inp["emb"][:, :, ts(vb, 128)])
            for t2 in range(NTC):
                if t2 == 0 and vb < NPRE:
                    continue
                logit_block(vb, t2, ebt)


# ------------------------------------------------------------------
# Host side
# ------------------------------------------------------------------

def _bf(a):
    import ml_dtypes

    return np.asarray(a, np.float32).astype(ml_dtypes.bfloat16)


def _kfold(w):
    """[in, out] -> [128, in//128, out] K-tiled layout."""
    i, o = w.shape
    return np.ascontiguousarray(
        w.reshape(i // 128, 128, o).transpose(1, 0, 2))


def _cols(v):
    """[n] -> [128, n//128] per-partition bias columns."""
    return np.ascontiguousarray(v.reshape(-1, 128).T)


def prep_inputs(inputs):
    """Full inputs -> list of 8 per-core input maps."""
    f = lambda a: np.asarray(a, np.float32)
    tokens = np.asarray(inputs["tokens"])
    tok_emb, pos_emb = f(inputs["tok_emb"]), f(inputs["pos_emb"])
    ln1_g, ln1_b = f(inputs["ln1_g"]), f(inputs["ln1_b"])
    wq, wk = f(inputs["wq"]), f(inputs["wk"])
    wv, wo = f(inputs["wv"]), f(inputs["wo"])
    ln2_g, ln2_b = f(inputs["ln2_g"]), f(inputs["ln2_b"])
    w1, b1 = f(inputs["w1"]), f(inputs["b1"])
    w2, b2 = f(inputs["w2"]), f(inputs["b2"])
    lnf_g = f(inputs["lnf_g"])

    sc = 1.0 / np.sqrt(HD)
    x0 = tok_emb[tokens] + pos_emb[:S][None]          # [B, S, D]
    ones = np.ones((128, 1), np.float32)
    ident = np.eye(128, dtype=np.float32)
    tri = np.triu(np.full((128, 128), MASK_VAL, np.float32), k=1)

    in_maps = []
    for core in range(N_CORES):
        b = core // TP
        tpr = core % TP
        m = {
            "x0": _kfold(np.ascontiguousarray(x0[b].T)).astype(np.float32),
            "ones": _bf(ones), "ident": _bf(ident), "tri": _bf(tri),
            "ident2": _bf(np.vstack([np.eye(64), np.eye(64)])),
        }
        qs = slice(tpr * DQ, (tpr + 1) * DQ)
        fs = slice(tpr * FFC, (tpr + 1) * FFC)
        for l in range(L):
            wql = wq[l][:, qs] * sc
            wkl = wk[l][:, qs]
            wvl = wv[l][:, qs]
            m[_f("wq", l)] = _bf(_kfold(ln1_g[l][:, None] * wql))
            m[_f("wk", l)] = _bf(_kfold(ln1_g[l][:, None] * wkl))
            m[_f("wv", l)] = _bf(_kfold(ln1_g[l][:, None] * wvl))
            m[_f("wo", l)] = _bf(_kfold(wo[l][qs, :]))
            m[_f("w1", l)] = _bf(_kfold(ln2_g[l][:, None] * w1[l][:, fs]))
            m[_f("w2", l)] = _bf(_kfold(w2[l][fs, :]))
            m[_f("bqkv", l)] = np.concatenate(
                [_cols(ln1_b[l] @ wql), _cols(ln1_b[l] @ wkl),
                 _cols(ln1_b[l] @ wvl)], axis=1).astype(np.float32)
            m[_f("b1", l)] = _cols(b1[l][fs] + ln2_b[l] @ w1[l][:, fs]).astype(
                np.float32)
            m[_f("b2", l)] = _cols(b2[l]).astype(np.float32)
        v0 = tpr * VS
        v1 = min(v0 + VS, V)
        epad = np.zeros((D, VSP), np.float32)
        epad[:, :v1 - v0] = (tok_emb[v0:v1] * lnf_g[None, :]).T
        m["emb"] = _bf(_kfold(epad))
        in_maps.append(m)
    return in_maps


_CACHED = {}


def _get_program():
    if "nc" not in _CACHED:
        _CACHED["nc"] = build_program()
    return _CACHED["nc"]


def run(inputs, trace=False, **kw):
    nc = _get_program()
    in_maps = prep_inputs(inputs)
    return run_bass_kernel_spmd(nc, in_maps, list(range(N_CORES)),
                                trace=trace, **kw)


def assemble(results, inputs):
    """Per-core logits -> full [B, S, V] float32."""
    lnf_b = np.asarray(inputs["lnf_b"], np.float32)
    tok_emb = np.asarray(inputs["tok_emb"], np.float32)
    out = np.empty((B, S, V), np.float32)
    for b in range(B):
        parts = []
        for tpr in range(TP):
            v0 = tpr * VS
            v1 = min(v0 + VS, V)
            parts.append(results[b * TP + tpr]["logits"][:v1 - v0, :].T)
        out[b] = np.concatenate(parts, axis=1)
    if np.any(lnf_b):
        out += (tok_emb @ lnf_b)[None, None, :]
    return out


def kernel(**inputs):
    res = run(inputs)
    return assemble(res.results, inputs)


if __name__ == "__main__":
    print("building program...")
    build_program()
    print("build + compile OK")


# revision 14
# speedup vs baseline: 1.4179x; 1.4179x over previous
"""GPT forward pass on 8 Trainium2 NeuronCores (Bass/Tile), bf16 compute.

Model: B=2, S=1024, D=1024, H=16 heads (hd=64), L=6 layers, V=50257,
tied embedding head.

Sharding: DP2 x TP4. Cores 0-3 compute batch element 0, cores 4-7
batch element 1. Within each group of 4: attention is head-sharded
(4 heads/core), the FFN hidden dim is column/row sharded (1024/core),
and the tied logit matrix is vocab-sharded (12565 rows/core, padded
to 12800). Two AllReduces per layer (post-attention, post-FFN) over
each group of 4, token-chunked (2 x 512) so collectives overlap
compute.

On-device layout: activations are feature-major ("transposed"):
x[p, k, t] = X^T[128k + p, t]. All matmuls take weights [in, out] as
the stationary operand and activations [in, tokens] as the moving
operand, producing outputs feature-major with zero activation
transposes. LayerNorm gains/biases and the attention 1/sqrt(hd) scale
are folded into the adjacent weights on the host, so the device only
computes the pure normalization (x - mean) * rsqrt(var + eps), with
stats via ones-matmuls on the PE and per-token broadcasts via GpSimd.

All matmul datapaths are bf16 (weights cast on host, activations cast
on write); PSUM accumulation stays fp32, the residual stream stays
fp32 in SBUF, and the AllReduce payload is bf16.
"""

import sys

sys.path.insert(0, "/opt/trn_rl_repo")

import contextlib

import numpy as np

import concourse.bacc as bacc
import concourse.mybir as mybir
import concourse.tile as tile
from concourse.bass import ts
from concourse.bass_utils import run_bass_kernel_spmd

F32 = mybir.dt.float32
BF16 = mybir.dt.bfloat16
AF = mybir.ActivationFunctionType
ALU = mybir.AluOpType

# Model dims
B, S, D, H, L, V = 2, 1024, 1024, 16, 6, 50257
HD = D // H           # 64
DFF = 4 * D           # 4096
N_CORES = 8
TP = 4                # tensor-parallel group size
HPC = H // TP         # heads per core = 4
DQ = HPC * HD         # per-core qkv width = 256
FFC = DFF // TP       # per-core ffn hidden = 1024
KD = D // 128         # 8 feature tiles
T = S                 # tokens per core (one batch element)
TCH = 512             # token chunk for AR pipelining
NTC = T // TCH        # 2
VS = 12565            # vocab rows per core (last core: 12562)
VSP = 12800           # padded vocab rows per core
MASK_VAL = -60.0

REPLICA_GROUPS = [[0, 1, 2, 3], [4, 5, 6, 7]]


def _f(name, l=None):
    return name if l is None else f"{name}{l}"


def build_program():
    """Build the SPMD bass program (same instruction stream on all cores)."""
    nc = bacc.Bacc("TRN2", target_bir_lowering=False, debug=False,
                   enable_asserts=True, num_devices=N_CORES)

    inp = {}

    def dram_in(name, shape, dtype=BF16):
        inp[name] = nc.dram_tensor(name, shape, dtype, kind="ExternalInput").ap()
        return inp[name]

    dram_in("x0", [128, KD, T], F32)
    dram_in("ones", [128, 1])
    dram_in("ident", [128, 128])
    dram_in("tri", [128, 128])
    for l in range(L):
        dram_in(_f("wq", l), [128, KD, DQ])
        dram_in(_f("wk", l), [128, KD, DQ])
        dram_in(_f("wv", l), [128, KD, DQ])
        dram_in(_f("wo", l), [128, DQ // 128, D])
        dram_in(_f("w1", l), [128, KD, FFC])
        dram_in(_f("w2", l), [128, FFC // 128, D])
        dram_in(_f("bqkv", l), [128, 6], F32)
        dram_in(_f("b1", l), [128, FFC // 128], F32)
        dram_in(_f("b2", l), [128, KD], F32)
    dram_in("emb", [128, KD, VSP])
    logits = nc.dram_tensor("logits", [VSP, T], F32, kind="ExternalOutput").ap()

    with tile.TileContext(nc) as tc:
        _body(tc, inp, logits)
    nc.compile()
    return nc


def _body(tc, inp, logits):
    nc = tc.nc
    ctx = contextlib.ExitStack()
    with ctx:
        # --- SBUF pools (sizes are KB/partition) ---
        singles = ctx.enter_context(tc.tile_pool(name="singles", bufs=1))
        xp = ctx.enter_context(tc.tile_pool(name="xp", bufs=1))        # 32
        npool = ctx.enter_context(tc.tile_pool(name="npool", bufs=1))  # 16
        xbp = ctx.enter_context(tc.tile_pool(name="xbp", bufs=2))      # 16
        tmp = ctx.enter_context(tc.tile_pool(name="tmp", bufs=2))      # 5
        qkv = ctx.enter_context(tc.tile_pool(name="qkv", bufs=1))      # 12
        vh = ctx.enter_context(tc.tile_pool(name="vh", bufs=2))        # 4
        et = ctx.enter_context(tc.tile_pool(name="et", bufs=1))        # 8
        op = ctx.enter_context(tc.tile_pool(name="op", bufs=1))        # 4
        hp = ctx.enter_context(tc.tile_pool(name="hp", bufs=1))        # 8
        arr = ctx.enter_context(tc.tile_pool(name="arr", bufs=1))      # 8
        wts = ctx.enter_context(tc.tile_pool(name="wts", bufs=2))      # 6
        embp = ctx.enter_context(tc.tile_pool(name="embp", bufs=2))    # 8
        stat = ctx.enter_context(tc.tile_pool(name="stat", bufs=2))
        bcp = ctx.enter_context(tc.tile_pool(name="bcp", bufs=1))      # 8
        lout = ctx.enter_context(tc.tile_pool(name="lout", bufs=1))
        bias = ctx.enter_context(tc.tile_pool(name="bias", bufs=3))
        stg = ctx.enter_context(tc.tile_pool(name="stg", bufs=2))      # 3
        # --- PSUM pools (8 banks total) ---
        ps_mm = ctx.enter_context(tc.tile_pool(name="ps_mm", bufs=2, space="PSUM"))
        ps_sc = ctx.enter_context(tc.tile_pool(name="ps_sc", bufs=2, space="PSUM"))
        ps_tr = ctx.enter_context(tc.tile_pool(name="ps_tr", bufs=2, space="PSUM"))
        ps_o = ctx.enter_context(tc.tile_pool(name="ps_o", bufs=2, space="PSUM"))
        # --- DRAM (collective bounce) ---
        dram = ctx.enter_context(tc.tile_pool(name="dram", bufs=4, space="DRAM"))

        # --- constants / persistent ---
        ones_t = singles.tile([128, 1], BF16)
        nc.sync.dma_start(out=ones_t[:], in_=inp["ones"][:])
        ident_t = singles.tile([128, 128], BF16)
        nc.sync.dma_start(out=ident_t[:], in_=inp["ident"][:])
        tri_t = singles.tile([128, 128], BF16)
        nc.sync.dma_start(out=tri_t[:], in_=inp["tri"][:])
        eps_t = singles.tile([1, 1], F32)
        nc.vector.memset(eps_t[:], 1e-5)

        xt = xp.tile([128, KD, T], F32, tag="x")
        nc.sync.dma_start(out=xt[:], in_=inp["x0"][:])

        def layer_norm_chunk(src, dst, c):
            """dst[:,:,c] (bf16) = (src - mean) * rsqrt(var + eps)."""
            cs = ts(c, TCH)
            xb = xbp.tile([128, KD, TCH], BF16, tag="xb")
            for k in range(KD):
                nc.vector.tensor_scalar_add(xb[:, k, :], src[:, k, cs], 0.0)
            s1 = ps_sc.tile([1, TCH], F32, tag="sc")
            s2 = ps_sc.tile([1, TCH], F32, tag="sc")
            for k in range(KD):
                nc.tensor.matmul(s1[:], ones_t[:], xb[:, k, :],
                                 start=(k == 0), stop=(k == KD - 1))
            for k in range(KD):
                sq = tmp.tile([128, TCH], BF16, tag="tmp")
                nc.vector.tensor_tensor(
                    out=sq[:], in0=xb[:, k, :], in1=xb[:, k, :], op=ALU.mult)
                nc.tensor.matmul(s2[:], ones_t[:], sq[:],
                                 start=(k == 0), stop=(k == KD - 1))
            # finishing math on [1, TCH] rows
            m = stat.tile([1, TCH], F32, tag="sa")
            nc.vector.tensor_scalar_mul(m[:], s1[:], 1.0 / D)
            msq = stat.tile([1, TCH], F32, tag="sb")
            nc.vector.tensor_tensor(out=msq[:], in0=m[:], in1=m[:],
                                    op=ALU.mult)
            var = stat.tile([1, TCH], F32, tag="sb")
            nc.vector.scalar_tensor_tensor(
                out=var[:], in0=s2[:], scalar=1.0 / D, in1=msq[:],
                op0=ALU.mult, op1=ALU.subtract)
            rs = stat.tile([1, TCH], F32, tag="sb")
            nc.scalar.activation(rs[:], var[:], AF.Sqrt, bias=eps_t[:])
            nc.vector.reciprocal(rs[:], rs[:])
            a = stat.tile([1, TCH], F32, tag="sa")
            nc.vector.scalar_tensor_tensor(
                out=a[:], in0=m[:], scalar=-1.0, in1=rs[:],
                op0=ALU.mult, op1=ALU.mult)
            rB = bcp.tile([128, TCH], F32, tag="rB")
            nc.gpsimd.partition_broadcast(rB[:], rs[:])
            aB = bcp.tile([128, TCH], F32, tag="aB")
            nc.gpsimd.partition_broadcast(aB[:], a[:])
            # dst = src * rB + aB   (two DVE passes per k)
            for k in range(KD):
                t2 = tmp.tile([128, TCH], F32, tag="tmp2")
                nc.vector.tensor_tensor(
                    out=t2[:], in0=src[:, k, cs], in1=rB[:], op=ALU.mult)
                nc.vector.tensor_tensor(
                    out=dst[:, k, cs], in0=t2[:], in1=aB[:], op=ALU.add)

        def proj_chunk(dst, wname, n_src, mchunks, bias_t, bcol0, c):
            """dst[:, m, c-chunk] (bf16) = W^T @ n_src[c-chunk] + bias."""
            cs = ts(c, TCH)
            for m in range(mchunks):
                wstrip = wts.tile([128, KD, 128], BF16, tag="w")
                nc.sync.dma_start(out=wstrip[:],
                                  in_=inp[wname][:, :, ts(m, 128)])
                pt = ps_mm.tile([128, TCH], F32, tag="mm")
                for k in range(KD):
                    nc.tensor.matmul(pt[:], wstrip[:, k, :],
                                     n_src[:, k, cs],
                                     start=(k == 0), stop=(k == KD - 1))
                nc.scalar.activation(
                    dst[:, m, cs], pt[:], AF.Identity,
                    bias=bias_t[:, bcol0 + m:bcol0 + m + 1])

        def qkv_head_chunk(l, c, n_src, qT, kT, vT, bqkv_t, vhs2):
            """QKV projections for token chunk c, plus the v-transposes
            whose key blocks live in chunk c. Each [128,128] transpose
            covers both heads of a head-pair; the transposed values land
            in vhs2[mh] = [128, KD, 130] with an all-ones column at 64
            and 129 so the AV matmul also produces the softmax row sums
            (output row 64 of a 65-row result)."""
            proj_chunk(qT, _f("wq", l), n_src, DQ // 128, bqkv_t, 0, c)
            proj_chunk(kT, _f("wk", l), n_src, DQ // 128, bqkv_t, 2, c)
            proj_chunk(vT, _f("wv", l), n_src, DQ // 128, bqkv_t, 4, c)
            for mh in range(DQ // 128):
                for j in range(4 * c, 4 * c + 4):
                    tpv = ps_tr.tile([128, 128], BF16, tag="tr")
                    nc.tensor.matmul(tpv[:], vT[:, mh, ts(j, 128)],
                                     ident_t[:], is_transpose=True,
                                     start=True, stop=True)
                    nc.scalar.copy(vhs2[mh][:, j, 0:64], tpv[:, 0:64])
                    nc.scalar.copy(vhs2[mh][:, j, 65:129], tpv[:, 64:128])

        def attn_chunk(l, c, qT, kT, vhs2, oT, inject=None, inject_at=1):
            """Scores are computed pre-transposed: eT[k, q] = exp(qk^T+mask)
            written straight to SBUF by the Exp activation (k-tile is the
            stationary operand), so no per-block transposes of the probs
            are needed. The AV matmul contracts over keys and its 65th
            output row (ones column of vhs2) is the softmax denominator;
            o is normalized by a per-token reciprocal on the way out."""
            nkj = 4 * (c + 1)
            for h in range(HPC):
                if h == inject_at and inject is not None:
                    inject()
                pp = 64 * (h % 2)
                mh = h // 2
                hh = h % 2
                etile = et.tile([128, KD, TCH], BF16, tag="et")
                for kj in range(nkj):
                    q0 = max(0, 128 * kj - TCH * c)
                    diag = 128 * kj - TCH * c >= 0
                    spt = ps_sc.tile([128, 512], F32, tag="sc")
                    nc.tensor.matmul(
                        spt[:, q0:TCH],
                        kT[pp:pp + 64, mh, ts(kj, 128)],
                        qT[pp:pp + 64, mh, TCH * c + q0:TCH * (c + 1)],
                        start=True, stop=not diag)
                    if diag:
                        # causal mask for the diagonal block (tri is
                        # strictly-lower-triangular MASK_VAL in [k, q])
                        nc.tensor.matmul(spt[:, q0:q0 + 128], ident_t[:],
                                         tri_t[:], start=False, stop=True)
                    nc.scalar.activation(etile[:, kj, q0:TCH],
                                         spt[:, q0:TCH], AF.Exp)
                po = ps_o.tile([65, TCH], F32, tag="o")
                for kj in range(nkj):
                    lo = max(0, 128 * kj - TCH * c)
                    nc.tensor.matmul(po[:, lo:TCH],
                                     vhs2[mh][:, kj, 65 * hh:65 * hh + 65],
                                     etile[:, kj, lo:TCH],
                                     start=(kj == 0), stop=(kj == nkj - 1))
                rs = stat.tile([1, TCH], F32, tag="rs")
                nc.vector.reciprocal(rs[:], po[64:65, :])
                rB2 = bcp.tile([64, TCH], F32, tag="rb2")
                nc.gpsimd.partition_broadcast(rB2[:], rs[:])
                nc.vector.tensor_tensor(
                    out=oT[pp:pp + 64, mh, ts(c, TCH)],
                    in0=po[0:64, :], in1=rB2[:], op=ALU.mult)

        def mm_ar_chunk(wname, kchunks, src_tile, c, src_is_chunk):
            """out-partial = W^T @ src for chunk c -> bf16 AllReduce."""
            ar_in = dram.tile([128, KD, TCH], BF16, tag="dr")
            ar_out = dram.tile([128, KD, TCH], BF16, tag="dr")
            for m in range(KD):
                wstrip = wts.tile([128, KD, 128], BF16, tag="w")
                nc.sync.dma_start(out=wstrip[:, 0:kchunks, :],
                                  in_=inp[wname][:, :, ts(m, 128)])
                pt = ps_mm.tile([128, TCH], F32, tag="mm")
                for k in range(kchunks):
                    s = (src_tile[:, k, :] if src_is_chunk
                         else src_tile[:, k, ts(c, TCH)])
                    nc.tensor.matmul(pt[:], wstrip[:, k, :], s,
                                     start=(k == 0), stop=(k == kchunks - 1))
                st_ = stg.tile([128, TCH], BF16, tag="stg")
                nc.scalar.copy(st_[:], pt[:])
                nc.sync.dma_start(out=ar_in[:, m, :], in_=st_[:])
            nc.gpsimd.collective_compute(
                "AllReduce", ALU.add, replica_groups=REPLICA_GROUPS,
                ins=[ar_in.opt()], outs=[ar_out.opt()])
            return ar_out

        def make_vhs2(l):
            vhs2 = []
            for mh in range(DQ // 128):
                v2 = vh.tile([128, KD, 130], BF16, tag="vh",
                             name=f"vh{l}_{mh}")
                nc.vector.memset(v2[:, :, 64:65], 1.0)
                nc.vector.memset(v2[:, :, 129:130], 1.0)
                vhs2.append(v2)
            return vhs2

        # ---------------- prologue: LN1 + QKV of layer 0 ----------------
        n_cur = npool.tile([128, KD, T], BF16, tag="n")
        bqkv_t = bias.tile([128, 6], F32, tag="bias")
        nc.sync.dma_start(out=bqkv_t[:], in_=inp[_f("bqkv", 0)][:])
        qT = qkv.tile([128, DQ // 128, T], BF16, tag="qT")
        kT = qkv.tile([128, DQ // 128, T], BF16, tag="kT")
        vT = qkv.tile([128, DQ // 128, T], BF16, tag="vT")
        vhs2 = make_vhs2(0)
        for c in range(NTC):
            layer_norm_chunk(xt, n_cur, c)
            qkv_head_chunk(0, c, n_cur, qT, kT, vT, bqkv_t, vhs2)

        def ffn_w1(l, c, n2, b1_t):
            cs = ts(c, TCH)
            hT = hp.tile([128, FFC // 128, TCH], BF16, tag="h",
                         name=f"hT{l}_{c}")
            for m in range(FFC // 128):
                wstrip = wts.tile([128, KD, 128], BF16, tag="w",
                                  name=f"w1s{l}_{c}_{m}")
                nc.sync.dma_start(out=wstrip[:],
                                  in_=inp[_f("w1", l)][:, :, ts(m, 128)])
                pt = ps_mm.tile([128, TCH], F32, tag="mm", name=f"p1_{l}_{c}_{m}")
                for k in range(KD):
                    nc.tensor.matmul(pt[:], wstrip[:, k, :], n2[:, k, cs],
                                     start=(k == 0), stop=(k == KD - 1))
                nc.scalar.activation(hT[:, m, :], pt[:], AF.Gelu,
                                     bias=b1_t[:, m:m + 1])
            return hT

        def add_f_lnnext(c, ar_f, b2_t, n_next):
            cs = ts(c, TCH)
            art = arr.tile([128, KD, TCH], BF16, tag="arr", name=f"artf{c}")
            nc.sync.dma_start(out=art[:], in_=ar_f[c][:])
            for m in range(KD):
                nc.vector.scalar_tensor_tensor(
                    out=xt[:, m, cs], in0=art[:, m, :],
                    scalar=b2_t[:, m:m + 1],
                    in1=xt[:, m, cs],
                    op0=ALU.add, op1=ALU.add)
            layer_norm_chunk(xt, n_next, c)

        for l in range(L):
            # ---------------- attention + Wo + AR, chunk-pipelined ----------------
            oT = op.tile([128, DQ // 128, T], BF16, tag="oT")
            n2 = npool.tile([128, KD, T], BF16, tag="n")
            ar_a = []

            def add_a(c):
                art = arr.tile([128, KD, TCH], BF16, tag="arr",
                               name=f"arta{c}_{l}")
                nc.sync.dma_start(out=art[:], in_=ar_a[c][:])
                nc.vector.tensor_tensor(
                    out=xt[:, :, ts(c, TCH)],
                    in0=xt[:, :, ts(c, TCH)],
                    in1=art[:], op=ALU.add)

            def add_a_ln2_c0():
                add_a(0)
                layer_norm_chunk(xt, n2, 0)

            attn_chunk(l, 0, qT, kT, vhs2, oT)
            ar_a.append(mm_ar_chunk(_f("wo", l), DQ // 128, oT, 0, False))
            attn_chunk(l, 1, qT, kT, vhs2, oT, inject=add_a_ln2_c0)
            ar_a.append(mm_ar_chunk(_f("wo", l), DQ // 128, oT, 1, False))

            # ------- residual c1 + LN2(c1) overlap FFN(c0) -------
            b1_t = bias.tile([128, FFC // 128], F32, tag="bias")
            nc.sync.dma_start(out=b1_t[:], in_=inp[_f("b1", l)][:])
            b2_t = bias.tile([128, KD], F32, tag="bias")
            nc.sync.dma_start(out=b2_t[:], in_=inp[_f("b2", l)][:])
            add_a(1)
            layer_norm_chunk(xt, n2, 1)

            ar_f = []
            hT0 = ffn_w1(l, 0, n2, b1_t)
            ar_f.append(mm_ar_chunk(_f("w2", l), FFC // 128, hT0, 0, True))
            hT1 = ffn_w1(l, 1, n2, b1_t)

            n_next = npool.tile([128, KD, T], BF16, tag="n")
            ar_f.append(mm_ar_chunk(_f("w2", l), FFC // 128, hT1, 1, True))
            add_f_lnnext(0, ar_f, b2_t, n_next)

            if l < L - 1:
                bqkv_t = bias.tile([128, 6], F32, tag="bias")
                nc.sync.dma_start(out=bqkv_t[:], in_=inp[_f("bqkv", l + 1)][:])
                qT = qkv.tile([128, DQ // 128, T], BF16, tag="qT")
                kT = qkv.tile([128, DQ // 128, T], BF16, tag="kT")
                vT = qkv.tile([128, DQ // 128, T], BF16, tag="vT")
                vhs2 = make_vhs2(l + 1)
                qkv_head_chunk(l + 1, 0, n_next, qT, kT, vT, bqkv_t, vhs2)
            add_f_lnnext(1, ar_f, b2_t, n_next)
            if l < L - 1:
                qkv_head_chunk(l + 1, 1, n_next, qT, kT, vT, bqkv_t, vhs2)
            n_cur = n_next

        # after the loop, n_cur holds the final layer norm
        nf = n_cur

        # logits^T [VSP, T]: emb strip is the stationary operand, nf moves.
        # A t2=0-only prefix runs first: it depends only on chunk 0 of the
        # final layernorm, hiding the last AllReduce + LN tail of chunk 1.
        NPRE = 16

        def logit_block(vb, t2, ebt):
            pt = ps_mm.tile([128, TCH], F32, tag="mm",
                            name=f"plg{vb}_{t2}")
            for k in range(KD):
                nc.tensor.matmul(pt[:], ebt[:, k, :],
                                 nf[:, k, ts(t2, TCH)],
                                 start=(k == 0), stop=(k == KD - 1))
            lo = lout.tile([128, TCH], F32, tag="lo", name=f"lo{vb}_{t2}")
            nc.scalar.copy(lo[:], pt[:])
            nc.sync.dma_start(out=logits[ts(vb, 128), ts(t2, TCH)],
                              in_=lo[:])

        for vb in range(NPRE):
            ebt = embp.tile([128, KD, 128], BF16, tag="emb", name=f"ebA{vb}")
            nc.sync.dma_start(out=ebt[:], in_=inp["emb"][:, :, ts(vb, 128)])
            logit_block(vb, 0, ebt)
        for vb in range(VSP // 128):
            ebt = embp.tile([128, KD, 128], BF16, tag="emb", name=f"ebB{vb}")
            nc.sync.dma_start(out=ebt[:], in_=inp["emb"][:, :, ts(vb, 128)])
            for t2 in range(NTC):
                if t2 == 0 and vb < NPRE:
                    continue
                logit_block(vb, t2, ebt)


# ------------------------------------------------------------------
# Host side
# ------------------------------------------------------------------

def _bf(a):
    import ml_dtypes

    return np.asarray(a, np.float32).astype(ml_dtypes.bfloat16)


def _kfold(w):
    """[in, out] -> [128, in//128, out] K-tiled layout."""
    i, o = w.shape
    return np.ascontiguousarray(
        w.reshape(i // 128, 128, o).transpose(1, 0, 2))


def _cols(v):
    """[n] -> [128, n//128] per-partition bias columns."""
    return np.ascontiguousarray(v.reshape(-1, 128).T)


def prep_inputs(inputs):
    """Full inputs -> list of 8 per-core input maps."""
    f = lambda a: np.asarray(a, np.float32)
    tokens = np.asarray(inputs["tokens"])
    tok_emb, pos_emb = f(inputs["tok_emb"]), f(inputs["pos_emb"])
    ln1_g, ln1_b = f(inputs["ln1_g"]), f(inputs["ln1_b"])
    wq, wk = f(inputs["wq"]), f(inputs["wk"])
    wv, wo = f(inputs["wv"]), f(inputs["wo"])
    ln2_g, ln2_b = f(inputs["ln2_g"]), f(inputs["ln2_b"])
    w1, b1 = f(inputs["w1"]), f(inputs["b1"])
    w2, b2 = f(inputs["w2"]), f(inputs["b2"])
    lnf_g = f(inputs["lnf_g"])

    sc = 1.0 / np.sqrt(HD)
    x0 = tok_emb[tokens] + pos_emb[:S][None]          # [B, S, D]
    ones = np.ones((128, 1), np.float32)
    ident = np.eye(128, dtype=np.float32)
    # strictly-lower-triangular mask in [key, query] layout
    tri = np.tril(np.full((128, 128), MASK_VAL, np.float32), k=-1)

    in_maps = []
    for core in range(N_CORES):
        b = core // TP
        tpr = core % TP
        m = {
            "x0": _kfold(np.ascontiguousarray(x0[b].T)).astype(np.float32),
            "ones": _bf(ones), "ident": _bf(ident), "tri": _bf(tri),
        }
        qs = slice(tpr * DQ, (tpr + 1) * DQ)
        fs = slice(tpr * FFC, (tpr + 1) * FFC)
        for l in range(L):
            wql = wq[l][:, qs] * sc
            wkl = wk[l][:, qs]
            wvl = wv[l][:, qs]
            m[_f("wq", l)] = _bf(_kfold(ln1_g[l][:, None] * wql))
            m[_f("wk", l)] = _bf(_kfold(ln1_g[l][:, None] * wkl))
            m[_f("wv", l)] = _bf(_kfold(ln1_g[l][:, None] * wvl))
            m[_f("wo", l)] = _bf(_kfold(wo[l][qs, :]))
            m[_f("w1", l)] = _bf(_kfold(ln2_g[l][:, None] * w1[l][:, fs]))
            m[_f("w2", l)] = _bf(_kfold(w2[l][fs, :]))
            m[_f("bqkv", l)] = np.concatenate(
                [_cols(ln1_b[l] @ wql), _cols(ln1_b[l] @ wkl),
                 _cols(ln1_b[l] @ wvl)], axis=1).astype(np.float32)
            m[_f("b1", l)] = _cols(b1[l][fs] + ln2_b[l] @ w1[l][:, fs]).astype(
                np.float32)
            m[_f("b2", l)] = _cols(b2[l]).astype(np.float32)
        v0 = tpr * VS
        v1 = min(v0 + VS, V)
        epad = np.zeros((D, VSP), np.float32)
        epad[:, :v1 - v0] = (tok_emb[v0:v1] * lnf_g[None, :]).T
        m["emb"] = _bf(_kfold(epad))
        in_maps.append(m)
    return in_maps


_CACHED = {}


def _get_program():
    if "nc" not in _CACHED:
        _CACHED["nc"] = build_program()
    return _CACHED["nc"]


def run(inputs, trace=False, **kw):
    nc = _get_program()
    in_maps = prep_inputs(inputs)
    return run_bass_kernel_spmd(nc, in_maps, list(range(N_CORES)),
                                trace=trace, **kw)


def assemble(results, inputs):
    """Per-core logits -> full [B, S, V] float32."""
    lnf_b = np.asarray(inputs["lnf_b"], np.float32)
    tok_emb = np.asarray(inputs["tok_emb"], np.float32)
    out = np.empty((B, S, V), np.float32)
    for b in range(B):
        parts = []
        for tpr in range(TP):
            v0 = tpr * VS
            v1 = min(v0 + VS, V)
            parts.append(results[b * TP + tpr]["logits"][:v1 - v0, :].T)
        out[b] = np.concatenate(parts, axis=1)
    if np.any(lnf_b):
        out += (tok_emb @ lnf_b)[None, None, :]
    return out


def kernel(**inputs):
    res = run(inputs)
    return assemble(res.results, inputs)


if __name__ == "__main__":
    print("building program...")
    build_program()
    print("build + compile OK")


# revision 29
# speedup vs baseline: 1.8262x; 1.2880x over previous
"""GPT forward pass on 8 Trainium2 NeuronCores (Bass/Tile), bf16 compute.

Model: B=2, S=1024, D=1024, H=16 heads (hd=64), L=6 layers, V=50257,
tied embedding head.

Sharding: DP2 x TP4. Cores 0-3 compute batch element 0, cores 4-7
batch element 1. Within each group of 4: attention is head-sharded
(4 heads/core), the FFN hidden dim is column/row sharded (1024/core),
and the tied logit matrix is vocab-sharded (12565 rows/core, padded
to 12800). Two AllReduces per layer (post-attention, post-FFN) over
each group of 4, token-chunked (2 x 512) so collectives overlap
compute.

On-device layout: activations are feature-major ("transposed"):
x[p, k, t] = X^T[128k + p, t]. All matmuls take weights [in, out] as
the stationary operand and activations [in, tokens] as the moving
operand, producing outputs feature-major with zero activation
transposes. LayerNorm gains/biases and the attention 1/sqrt(hd) scale
are folded into the adjacent weights on the host, so the device only
computes the pure normalization (x - mean) * rsqrt(var + eps), with
stats via ones-matmuls on the PE and per-token broadcasts via GpSimd.

All matmul datapaths are bf16 (weights cast on host, activations cast
on write); PSUM accumulation stays fp32, the residual stream stays
fp32 in SBUF, and the AllReduce payload is bf16.
"""

import sys

sys.path.insert(0, "/opt/trn_rl_repo")

import contextlib

import numpy as np

import concourse.bacc as bacc
import concourse.mybir as mybir
import concourse.tile as tile
from concourse.bass import ts
from concourse.bass_utils import run_bass_kernel_spmd

F32 = mybir.dt.float32
BF16 = mybir.dt.bfloat16
AF = mybir.ActivationFunctionType
ALU = mybir.AluOpType

# Model dims
B, S, D, H, L, V = 2, 1024, 1024, 16, 6, 50257
HD = D // H           # 64
DFF = 4 * D           # 4096
N_CORES = 8
TP = 4                # tensor-parallel group size
HPC = H // TP         # heads per core = 4
DQ = HPC * HD         # per-core qkv width = 256
FFC = DFF // TP       # per-core ffn hidden = 1024
KD = D // 128         # 8 feature tiles
T = S                 # tokens per core (one batch element)
TCH = 512             # token chunk for AR pipelining
NTC = T // TCH        # 2
VS = 12565            # vocab rows per core (last core: 12562)
VSP = 12800           # padded vocab rows per core
MASK_VAL = -60.0

REPLICA_GROUPS = [[0, 1, 2, 3], [4, 5, 6, 7]]


def _f(name, l=None):
    return name if l is None else f"{name}{l}"


def build_program():
    """Build the SPMD bass program (same instruction stream on all cores)."""
    nc = bacc.Bacc("TRN2", target_bir_lowering=False, debug=False,
                   enable_asserts=True, num_devices=N_CORES)

    inp = {}

    def dram_in(name, shape, dtype=BF16):
        inp[name] = nc.dram_tensor(name, shape, dtype, kind="ExternalInput").ap()
        return inp[name]

    dram_in("x0", [128, KD, T])
    dram_in("ones", [128, 1])
    dram_in("ident", [128, 128])
    dram_in("tri", [128, 128])
    for l in range(L):
        dram_in(_f("wq", l), [128, KD, DQ])
        dram_in(_f("wk", l), [128, KD, DQ])
        dram_in(_f("wv", l), [128, KD, DQ])
        dram_in(_f("wo", l), [128, DQ // 128, D])
        dram_in(_f("w1", l), [128, KD, FFC])
        dram_in(_f("w2", l), [128, FFC // 128, D])
        dram_in(_f("bqkv", l), [128, 6], F32)
        dram_in(_f("b1", l), [128, FFC // 128], F32)
        dram_in(_f("b2", l), [128, KD], F32)
    dram_in("emb", [128, KD, VSP])
    logits = nc.dram_tensor("logits", [VSP, T], F32, kind="ExternalOutput").ap()

    with tile.TileContext(nc) as tc:
        _body(tc, inp, logits)
    nc.compile()
    return nc


def _body(tc, inp, logits):
    nc = tc.nc
    ctx = contextlib.ExitStack()
    with ctx:
        # --- SBUF pools (sizes are KB/partition) ---
        singles = ctx.enter_context(tc.tile_pool(name="singles", bufs=1))
        xp = ctx.enter_context(tc.tile_pool(name="xp", bufs=1))        # 16
        npool = ctx.enter_context(tc.tile_pool(name="npool", bufs=1))  # 16
        tmp = ctx.enter_context(tc.tile_pool(name="tmp", bufs=2))      # 6
        qkv = ctx.enter_context(tc.tile_pool(name="qkv", bufs=1))      # 12
        vh = ctx.enter_context(tc.tile_pool(name="vh", bufs=2))        # 4
        et = ctx.enter_context(tc.tile_pool(name="et", bufs=2))        # 16
        op = ctx.enter_context(tc.tile_pool(name="op", bufs=1))        # 4
        hp = ctx.enter_context(tc.tile_pool(name="hp", bufs=2))        # 16
        arr = ctx.enter_context(tc.tile_pool(name="arr", bufs=1))      # 8
        wts = ctx.enter_context(tc.tile_pool(name="wts", bufs=2))      # 32
        wff = ctx.enter_context(tc.tile_pool(name="wff", bufs=1))      # 32
        embp = ctx.enter_context(tc.tile_pool(name="embp", bufs=2))    # 8
        stat = ctx.enter_context(tc.tile_pool(name="stat", bufs=2))
        bcp = ctx.enter_context(tc.tile_pool(name="bcp", bufs=1))      # 8
        lout = ctx.enter_context(tc.tile_pool(name="lout", bufs=1))
        bias = ctx.enter_context(tc.tile_pool(name="bias", bufs=3))
        stg = ctx.enter_context(tc.tile_pool(name="stg", bufs=2))      # 3
        # --- PSUM pools (8 banks total) ---
        ps_mm = ctx.enter_context(tc.tile_pool(name="ps_mm", bufs=2, space="PSUM"))
        ps_sc = ctx.enter_context(tc.tile_pool(name="ps_sc", bufs=2, space="PSUM"))
        ps_st = ctx.enter_context(tc.tile_pool(name="ps_st", bufs=1, space="PSUM"))
        ps_tr = ctx.enter_context(tc.tile_pool(name="ps_tr", bufs=1, space="PSUM"))
        ps_o = ctx.enter_context(tc.tile_pool(name="ps_o", bufs=2, space="PSUM"))
        # --- DRAM (collective bounce) ---
        dram = ctx.enter_context(tc.tile_pool(name="dram", bufs=4, space="DRAM"))

        # --- constants / persistent ---
        ones_t = singles.tile([128, 1], BF16)
        nc.sync.dma_start(out=ones_t[:], in_=inp["ones"][:])
        ident_t = singles.tile([128, 128], BF16)
        nc.sync.dma_start(out=ident_t[:], in_=inp["ident"][:])
        tri_t = singles.tile([128, 128], BF16)
        nc.sync.dma_start(out=tri_t[:], in_=inp["tri"][:])
        eps_t = singles.tile([1, 1], F32)
        nc.vector.memset(eps_t[:], 1e-5)

        xt = xp.tile([128, KD, T], BF16, tag="x")
        nc.sync.dma_start(out=xt[:], in_=inp["x0"][:])

        def layer_norm_chunk(src, dst, c):
            """dst[:,:,c] (bf16) = (src - mean) * rsqrt(var + eps)."""
            cs = ts(c, TCH)
            s12 = ps_st.tile([33, TCH], F32, tag="st")
            s1 = s12[0:1, :]
            s2 = s12[32:33, :]
            for k in range(KD):
                nc.tensor.matmul(s1[:], ones_t[:], src[:, k, cs],
                                 start=(k == 0), stop=(k == KD - 1))
            for k in range(KD):
                sq = tmp.tile([128, TCH], BF16, tag="tmp")
                nc.vector.tensor_tensor(
                    out=sq[:], in0=src[:, k, cs], in1=src[:, k, cs],
                    op=ALU.mult)
                nc.tensor.matmul(s2[:], ones_t[:], sq[:],
                                 start=(k == 0), stop=(k == KD - 1))
            # finishing math on [1, TCH] rows
            m = stat.tile([1, TCH], F32, tag="sa")
            nc.vector.tensor_scalar_mul(m[:], s1[:], 1.0 / D)
            msq = stat.tile([1, TCH], F32, tag="sb")
            nc.vector.tensor_tensor(out=msq[:], in0=m[:], in1=m[:],
                                    op=ALU.mult)
            var = stat.tile([1, TCH], F32, tag="sb")
            nc.vector.scalar_tensor_tensor(
                out=var[:], in0=s2[:], scalar=1.0 / D, in1=msq[:],
                op0=ALU.mult, op1=ALU.subtract)
            sd = stat.tile([1, TCH], F32, tag="sb")
            nc.scalar.activation(sd[:], var[:], AF.Sqrt, bias=eps_t[:])
            rs = stat.tile([1, TCH], F32, tag="sb")
            nc.vector.reciprocal_approx_fast(rs[:], sd[:])
            rB = bcp.tile([128, TCH], F32, tag="rB")
            nc.gpsimd.partition_broadcast(rB[:], rs[:])
            mB = bcp.tile([128, TCH], F32, tag="mB")
            nc.gpsimd.partition_broadcast(mB[:], m[:])
            # dst = (src - mB) * rB   (two DVE passes per k)
            for k in range(KD):
                t2 = tmp.tile([128, TCH], F32, tag="tmp2")
                nc.vector.tensor_tensor(
                    out=t2[:], in0=src[:, k, cs], in1=mB[:], op=ALU.subtract)
                nc.vector.tensor_tensor(
                    out=dst[:, k, cs], in0=t2[:], in1=rB[:], op=ALU.mult)

        def proj_chunk(dst, wt, n_src, mchunks, bias_t, bcol0, c):
            """dst[:, m, c-chunk] (bf16) = W^T @ n_src[c-chunk] + bias."""
            cs = ts(c, TCH)
            for m in range(mchunks):
                pt = ps_mm.tile([128, TCH], F32, tag="mm")
                for k in range(KD):
                    nc.tensor.matmul(pt[:], wt[:, k, ts(m, 128)],
                                     n_src[:, k, cs],
                                     start=(k == 0), stop=(k == KD - 1))
                nc.scalar.activation(
                    dst[:, m, cs], pt[:], AF.Identity,
                    bias=bias_t[:, bcol0 + m:bcol0 + m + 1])

        def qkv_head_chunk(c, n_src, qT, kT, vT, wq_t, wk_t, wv_t, bqkv_t,
                           vhs2):
            """QKV projections for token chunk c, plus the v-transposes
            whose key blocks live in chunk c. Each [128,128] transpose
            covers both heads of a head-pair; the transposed values land
            in vhs2[mh] = [128, KD, 130] with an all-ones column at 64
            and 129 so the AV matmul also produces the softmax row sums
            (output row 64 of a 65-row result)."""
            proj_chunk(qT, wq_t, n_src, DQ // 128, bqkv_t, 0, c)
            proj_chunk(kT, wk_t, n_src, DQ // 128, bqkv_t, 2, c)
            proj_chunk(vT, wv_t, n_src, DQ // 128, bqkv_t, 4, c)
            for mh in range(DQ // 128):
                for j in range(4 * c, 4 * c + 4):
                    tpv = ps_tr.tile([128, 128], BF16, tag="tr")
                    nc.tensor.matmul(tpv[:], vT[:, mh, ts(j, 128)],
                                     ident_t[:], is_transpose=True,
                                     start=True, stop=True)
                    nc.scalar.copy(vhs2[mh][:, j, 0:64], tpv[:, 0:64])
                    nc.vector.tensor_scalar_add(vhs2[mh][:, j, 65:129],
                                                tpv[:, 64:128], 0.0)

        def attn_chunk(l, c, qT, kT, vhs2, oT, inject=None, inject_at=1):
            """Scores are computed pre-transposed: eT[k, q] = exp(qk^T+mask)
            written straight to SBUF by the Exp activation (k-tile is the
            stationary operand), so no per-block transposes of the probs
            are needed. The AV matmul contracts over keys and its 65th
            output row (ones column of vhs2) is the softmax denominator;
            o is normalized by a per-token reciprocal on the way out."""
            nkj = 4 * (c + 1)
            for h in range(HPC):
                if h == inject_at and inject is not None:
                    inject()
                pp = 64 * (h % 2)
                mh = h // 2
                hh = h % 2
                etile = et.tile([128, KD, TCH], BF16, tag="et")
                for kj in range(nkj):
                    q0 = max(0, 128 * kj - TCH * c)
                    diag = 128 * kj - TCH * c >= 0
                    spt = ps_sc.tile([128, 512], F32, tag="sc")
                    nc.tensor.matmul(
                        spt[:, q0:TCH],
                        kT[pp:pp + 64, mh, ts(kj, 128)],
                        qT[pp:pp + 64, mh, TCH * c + q0:TCH * (c + 1)],
                        start=True, stop=not diag)
                    if diag:
                        # causal mask for the diagonal block (tri is
                        # strictly-lower-triangular MASK_VAL in [k, q])
                        nc.tensor.matmul(spt[:, q0:q0 + 128], ident_t[:],
                                         tri_t[:], start=False, stop=True)
                    nc.scalar.activation(etile[:, kj, q0:TCH],
                                         spt[:, q0:TCH], AF.Exp)
                po = ps_o.tile([65, TCH], F32, tag="o")
                for kj in range(nkj):
                    lo = max(0, 128 * kj - TCH * c)
                    nc.tensor.matmul(po[:, lo:TCH],
                                     vhs2[mh][:, kj, 65 * hh:65 * hh + 65],
                                     etile[:, kj, lo:TCH],
                                     start=(kj == 0), stop=(kj == nkj - 1))
                rsS = stat.tile([1, TCH], F32, tag="rsS")
                nc.vector.tensor_scalar_add(rsS[:], po[64:65, :], 0.0)
                rs = stat.tile([1, TCH], F32, tag="rs")
                nc.vector.reciprocal_approx_fast(rs[:], rsS[:])
                rB2 = bcp.tile([64, TCH], F32, tag="rb2")
                nc.gpsimd.partition_broadcast(rB2[:], rs[:])
                nc.vector.tensor_tensor(
                    out=oT[pp:pp + 64, mh, ts(c, TCH)],
                    in0=po[0:64, :], in1=rB2[:], op=ALU.mult)

        def mm_ar_chunk(wt, kchunks, src_tile, c, src_is_chunk):
            """out-partial = W^T @ src for chunk c -> bf16 AllReduce."""
            ar_in = dram.tile([128, KD, TCH], BF16, tag="dr")
            ar_out = dram.tile([128, KD, TCH], BF16, tag="dr")
            for m in range(KD):
                pt = ps_mm.tile([128, TCH], F32, tag="mm")
                for k in range(kchunks):
                    s = (src_tile[:, k, :] if src_is_chunk
                         else src_tile[:, k, ts(c, TCH)])
                    nc.tensor.matmul(pt[:], wt[:, k, ts(m, 128)], s,
                                     start=(k == 0), stop=(k == kchunks - 1))
                st_ = stg.tile([128, TCH], BF16, tag="stg")
                nc.scalar.copy(st_[:], pt[:])
                nc.sync.dma_start(out=ar_in[:, m, :], in_=st_[:])
            nc.gpsimd.collective_compute(
                "AllReduce", ALU.add, replica_groups=REPLICA_GROUPS,
                ins=[ar_in.opt()], outs=[ar_out.opt()])
            return ar_out

        def load_w(name, kd, width, pool_tag, bufs_pool):
            wt = bufs_pool.tile([128, kd, width], BF16, tag=pool_tag,
                                name=f"{name}_t")
            nc.sync.dma_start(out=wt[:], in_=inp[name][:])
            return wt

        def make_vhs2(l):
            vhs2 = []
            for mh in range(DQ // 128):
                v2 = vh.tile([128, KD, 130], BF16, tag="vh",
                             name=f"vh{l}_{mh}")
                nc.vector.memset(v2[:, :, 64:65], 1.0)
                nc.vector.memset(v2[:, :, 129:130], 1.0)
                vhs2.append(v2)
            return vhs2

        # ---------------- prologue: LN1 + QKV of layer 0 ----------------
        n_cur = npool.tile([128, KD, T], BF16, tag="n")
        bqkv_t = bias.tile([128, 6], F32, tag="bias")
        nc.sync.dma_start(out=bqkv_t[:], in_=inp[_f("bqkv", 0)][:])
        wq_t = load_w(_f("wq", 0), KD, DQ, "wq", wts)
        wk_t = load_w(_f("wk", 0), KD, DQ, "wk", wts)
        wv_t = load_w(_f("wv", 0), KD, DQ, "wv", wts)
        wo_t = load_w(_f("wo", 0), DQ // 128, D, "wo", wts)
        qT = qkv.tile([128, DQ // 128, T], BF16, tag="qT")
        kT = qkv.tile([128, DQ // 128, T], BF16, tag="kT")
        vT = qkv.tile([128, DQ // 128, T], BF16, tag="vT")
        vhs2 = make_vhs2(0)
        for c in range(NTC):
            layer_norm_chunk(xt, n_cur, c)
            qkv_head_chunk(c, n_cur, qT, kT, vT, wq_t, wk_t, wv_t, bqkv_t,
                           vhs2)

        def ffn_w1(l, c, n2, w1_t, b1_t):
            cs = ts(c, TCH)
            hT = hp.tile([128, FFC // 128, TCH], BF16, tag="h",
                         name=f"hT{l}_{c}")
            for m in range(FFC // 128):
                pt = ps_mm.tile([128, TCH], F32, tag="mm", name=f"p1_{l}_{c}_{m}")
                for k in range(KD):
                    nc.tensor.matmul(pt[:], w1_t[:, k, ts(m, 128)],
                                     n2[:, k, cs],
                                     start=(k == 0), stop=(k == KD - 1))
                nc.scalar.activation(hT[:, m, :], pt[:], AF.Gelu,
                                     bias=b1_t[:, m:m + 1])
            return hT

        def add_f_lnnext(c, ar_f, b2_t, n_next):
            cs = ts(c, TCH)
            art = arr.tile([128, KD, TCH], BF16, tag="arr", name=f"artf{c}")
            nc.sync.dma_start(out=art[:], in_=ar_f[c][:])
            for m in range(KD):
                nc.vector.scalar_tensor_tensor(
                    out=xt[:, m, cs], in0=art[:, m, :],
                    scalar=b2_t[:, m:m + 1],
                    in1=xt[:, m, cs],
                    op0=ALU.add, op1=ALU.add)
            layer_norm_chunk(xt, n_next, c)

        for l in range(L):
            # ---------------- attention + Wo + AR, chunk-pipelined ----------------
            w1_t = load_w(_f("w1", l), KD, FFC, "w1", wff)
            w2_t = load_w(_f("w2", l), FFC // 128, D, "w2", wff)
            oT = op.tile([128, DQ // 128, T], BF16, tag="oT")
            n2 = npool.tile([128, KD, T], BF16, tag="n")
            ar_a = []

            def add_a(c):
                art = arr.tile([128, KD, TCH], BF16, tag="arr",
                               name=f"arta{c}_{l}")
                nc.sync.dma_start(out=art[:], in_=ar_a[c][:])
                for k in range(KD):
                    nc.vector.tensor_tensor(
                        out=xt[:, k, ts(c, TCH)],
                        in0=xt[:, k, ts(c, TCH)],
                        in1=art[:, k, :], op=ALU.add)

            def add_a_ln2_c0():
                add_a(0)
                layer_norm_chunk(xt, n2, 0)

            attn_chunk(l, 0, qT, kT, vhs2, oT)
            ar_a.append(mm_ar_chunk(wo_t, DQ // 128, oT, 0, False))
            attn_chunk(l, 1, qT, kT, vhs2, oT, inject=add_a_ln2_c0)
            ar_a.append(mm_ar_chunk(wo_t, DQ // 128, oT, 1, False))

            # ------- residual c1 + LN2(c1) overlap FFN(c0) -------
            b1_t = bias.tile([128, FFC // 128], F32, tag="bias")
            nc.sync.dma_start(out=b1_t[:], in_=inp[_f("b1", l)][:])
            b2_t = bias.tile([128, KD], F32, tag="bias")
            nc.sync.dma_start(out=b2_t[:], in_=inp[_f("b2", l)][:])
            add_a(1)
            layer_norm_chunk(xt, n2, 1)

            ar_f = []
            hT0 = ffn_w1(l, 0, n2, w1_t, b1_t)
            ar_f.append(mm_ar_chunk(w2_t, FFC // 128, hT0, 0, True))
            hT1 = ffn_w1(l, 1, n2, w1_t, b1_t)

            n_next = npool.tile([128, KD, T], BF16, tag="n")
            ar_f.append(mm_ar_chunk(w2_t, FFC // 128, hT1, 1, True))
            add_f_lnnext(0, ar_f, b2_t, n_next)

            if l < L - 1:
                bqkv_t = bias.tile([128, 6], F32, tag="bias")
                nc.sync.dma_start(out=bqkv_t[:], in_=inp[_f("bqkv", l + 1)][:])
                wq_t = load_w(_f("wq", l + 1), KD, DQ, "wq", wts)
                wk_t = load_w(_f("wk", l + 1), KD, DQ, "wk", wts)
                wv_t = load_w(_f("wv", l + 1), KD, DQ, "wv", wts)
                wo_t = load_w(_f("wo", l + 1), DQ // 128, D, "wo", wts)
                qT = qkv.tile([128, DQ // 128, T], BF16, tag="qT")
                kT = qkv.tile([128, DQ // 128, T], BF16, tag="kT")
                vT = qkv.tile([128, DQ // 128, T], BF16, tag="vT")
                vhs2 = make_vhs2(l + 1)
                qkv_head_chunk(0, n_next, qT, kT, vT, wq_t, wk_t, wv_t,
                               bqkv_t, vhs2)
            add_f_lnnext(1, ar_f, b2_t, n_next)
            if l < L - 1:
                qkv_head_chunk(1, n_next, qT, kT, vT, wq_t, wk_t, wv_t,
                               bqkv_t, vhs2)
            n_cur = n_next

        # after the loop, n_cur holds the final layer norm
        nf = n_cur

        # logits^T [VSP, T]: emb strip is the stationary operand, nf moves.
        # A t2=0-only prefix runs first: it depends only on chunk 0 of the
        # final layernorm, hiding the last AllReduce + LN tail of chunk 1.
        NPRE = 16

        def logit_block(vb, t2, ebt):
            pt = ps_mm.tile([128, TCH], F32, tag="mm",
                            name=f"plg{vb}_{t2}")
            for k in range(KD):
                nc.tensor.matmul(pt[:], ebt[:, k, :],
                                 nf[:, k, ts(t2, TCH)],
                                 start=(k == 0), stop=(k == KD - 1))
            lo = lout.tile([128, TCH], F32, tag="lo", name=f"lo{vb}_{t2}")
            nc.scalar.copy(lo[:], pt[:])
            nc.sync.dma_start(out=logits[ts(vb, 128), ts(t2, TCH)],
                              in_=lo[:])

        for vb in range(NPRE):
            ebt = embp.tile([128, KD, 128], BF16, tag="emb", name=f"ebA{vb}")
            nc.sync.dma_start(out=ebt[:], in_=inp["emb"][:, :, ts(vb, 128)])
            logit_block(vb, 0, ebt)
        for vb in range(VSP // 128):
            ebt = embp.tile([128, KD, 128], BF16, tag="emb", name=f"ebB{vb}")
            nc.sync.dma_start(out=ebt[:], in_=inp["emb"][:, :, ts(vb, 128)])
            for t2 in range(NTC):
                if t2 == 0 and vb < NPRE:
                    continue
                logit_block(vb, t2, ebt)


# ------------------------------------------------------------------
# Host side
# ------------------------------------------------------------------

def _bf(a):
    import ml_dtypes

    return np.asarray(a, np.float32).astype(ml_dtypes.bfloat16)


def _kfold(w):
    """[in, out] -> [128, in//128, out] K-tiled layout."""
    i, o = w.shape
    return np.ascontiguousarray(
        w.reshape(i // 128, 128, o).transpose(1, 0, 2))


def _cols(v):
    """[n] -> [128, n//128] per-partition bias columns."""
    return np.ascontiguousarray(v.reshape(-1, 128).T)


def prep_inputs(inputs):
    """Full inputs -> list of 8 per-core input maps."""
    f = lambda a: np.asarray(a, np.float32)
    tokens = np.asarray(inputs["tokens"])
    tok_emb, pos_emb = f(inputs["tok_emb"]), f(inputs["pos_emb"])
    ln1_g, ln1_b = f(inputs["ln1_g"]), f(inputs["ln1_b"])
    wq, wk = f(inputs["wq"]), f(inputs["wk"])
    wv, wo = f(inputs["wv"]), f(inputs["wo"])
    ln2_g, ln2_b = f(inputs["ln2_g"]), f(inputs["ln2_b"])
    w1, b1 = f(inputs["w1"]), f(inputs["b1"])
    w2, b2 = f(inputs["w2"]), f(inputs["b2"])
    lnf_g = f(inputs["lnf_g"])

    sc = 1.0 / np.sqrt(HD)
    x0 = tok_emb[tokens] + pos_emb[:S][None]          # [B, S, D]
    ones = np.ones((128, 1), np.float32)
    ident = np.eye(128, dtype=np.float32)
    # strictly-lower-triangular mask in [key, query] layout
    tri = np.tril(np.full((128, 128), MASK_VAL, np.float32), k=-1)

    in_maps = []
    for core in range(N_CORES):
        b = core // TP
        tpr = core % TP
        m = {
            "x0": _bf(_kfold(np.ascontiguousarray(x0[b].T))),
            "ones": _bf(ones), "ident": _bf(ident), "tri": _bf(tri),
        }
        qs = slice(tpr * DQ, (tpr + 1) * DQ)
        fs = slice(tpr * FFC, (tpr + 1) * FFC)
        for l in range(L):
            wql = wq[l][:, qs] * sc
            wkl = wk[l][:, qs]
            wvl = wv[l][:, qs]
            m[_f("wq", l)] = _bf(_kfold(ln1_g[l][:, None] * wql))
            m[_f("wk", l)] = _bf(_kfold(ln1_g[l][:, None] * wkl))
            m[_f("wv", l)] = _bf(_kfold(ln1_g[l][:, None] * wvl))
            m[_f("wo", l)] = _bf(_kfold(wo[l][qs, :]))
            m[_f("w1", l)] = _bf(_kfold(ln2_g[l][:, None] * w1[l][:, fs]))
            m[_f("w2", l)] = _bf(_kfold(w2[l][fs, :]))
            m[_f("bqkv", l)] = np.concatenate(
                [_cols(ln1_b[l] @ wql), _cols(ln1_b[l] @ wkl),
                 _cols(ln1_b[l] @ wvl)], axis=1).astype(np.float32)
            m[_f("b1", l)] = _cols(b1[l][fs] + ln2_b[l] @ w1[l][:, fs]).astype(
                np.float32)
            m[_f("b2", l)] = _cols(b2[l]).astype(np.float32)
        v0 = tpr * VS
        v1 = min(v0 + VS, V)
        epad = np.zeros((D, VSP), np.float32)
        epad[:, :v1 - v0] = (tok_emb[v0:v1] * lnf_g[None, :]).T
        m["emb"] = _bf(_kfold(epad))
        in_maps.append(m)
    return in_maps


_CACHED = {}


def _get_program():
    if "nc" not in _CACHED:
        _CACHED["nc"] = build_program()
    return _CACHED["nc"]


def run(inputs, trace=False, **kw):
    nc = _get_program()
    in_maps = prep_inputs(inputs)
    return run_bass_kernel_spmd(nc, in_maps, list(range(N_CORES)),
                                trace=trace, **kw)


def assemble(results, inputs):
    """Per-core logits -> full [B, S, V] float32."""
    lnf_b = np.asarray(inputs["lnf_b"], np.float32)
    tok_emb = np.asarray(inputs["tok_emb"], np.float32)
    out = np.empty((B, S, V), np.float32)
    for b in range(B):
        parts = []
        for tpr in range(TP):
            v0 = tpr * VS
            v1 = min(v0 + VS, V)
            parts.append(results[b * TP + tpr]["logits"][:v1 - v0, :].T)
        out[b] = np.concatenate(parts, axis=1)
    if np.any(lnf_b):
        out += (tok_emb @ lnf_b)[None, None, :]
    return out


def kernel(**inputs):
    res = run(inputs)
    return assemble(res.results, inputs)


if __name__ == "__main__":
    print("building program...")
    build_program()
    print("build + compile OK")


# revision 35
# speedup vs baseline: 2.1273x; 1.1649x over previous
"""GPT forward pass on 8 Trainium2 NeuronCores (Bass/Tile), bf16 compute.

Model: B=2, S=1024, D=1024, H=16 heads (hd=64), L=6 layers, V=50257,
tied embedding head.

Sharding: DP2 x TP4. Cores 0-3 compute batch element 0, cores 4-7
batch element 1. Within each group of 4: attention is head-sharded
(4 heads/core), the FFN hidden dim is column/row sharded (1024/core),
and the tied logit matrix is vocab-sharded (12565 rows/core, padded
to 12800). Two AllReduces per layer (post-attention, post-FFN) over
each group of 4, token-chunked (2 x 512) so collectives overlap
compute.

On-device layout: activations are feature-major ("transposed"):
x[p, k, t] = X^T[128k + p, t]. All matmuls take weights [in, out] as
the stationary operand and activations [in, tokens] as the moving
operand, producing outputs feature-major with zero activation
transposes. LayerNorm gains/biases and the attention 1/sqrt(hd) scale
are folded into the adjacent weights on the host, so the device only
computes the pure normalization (x - mean) * rsqrt(var + eps), with
stats via ones-matmuls on the PE and per-token broadcasts via GpSimd.

All matmul datapaths are bf16 (weights cast on host, activations cast
on write); PSUM accumulation stays fp32, the residual stream stays
fp32 in SBUF, and the AllReduce payload is bf16.
"""

import sys

sys.path.insert(0, "/opt/trn_rl_repo")

import contextlib

import numpy as np

import concourse.bacc as bacc
import concourse.mybir as mybir
import concourse.tile as tile
from concourse.bass import ts
from concourse.bass_utils import run_bass_kernel_spmd

F32 = mybir.dt.float32
BF16 = mybir.dt.bfloat16
AF = mybir.ActivationFunctionType
ALU = mybir.AluOpType

# Model dims
B, S, D, H, L, V = 2, 1024, 1024, 16, 6, 50257
HD = D // H           # 64
DFF = 4 * D           # 4096
N_CORES = 8
TP = 4                # tensor-parallel group size
HPC = H // TP         # heads per core = 4
DQ = HPC * HD         # per-core qkv width = 256
FFC = DFF // TP       # per-core ffn hidden = 1024
KD = D // 128         # 8 feature tiles
T = S                 # tokens per core (one batch element)
TCH = 512             # token chunk for AR pipelining
NTC = T // TCH        # 2
VS = 12565            # vocab rows per core (last core: 12562)
VSP = 12800           # padded vocab rows per core
MASK_VAL = -60.0

REPLICA_GROUPS = [[0, 1, 2, 3], [4, 5, 6, 7]]


def _f(name, l=None):
    return name if l is None else f"{name}{l}"


def build_program():
    """Build the SPMD bass program (same instruction stream on all cores)."""
    nc = bacc.Bacc("TRN2", target_bir_lowering=False, debug=False,
                   enable_asserts=True, num_devices=N_CORES)

    inp = {}

    def dram_in(name, shape, dtype=BF16):
        inp[name] = nc.dram_tensor(name, shape, dtype, kind="ExternalInput").ap()
        return inp[name]

    dram_in("x0", [128, KD, T])
    dram_in("ones", [128, 1])
    dram_in("ident", [128, 128])
    dram_in("tri", [128, 128])
    for l in range(L):
        dram_in(_f("wq", l), [128, KD, DQ])
        dram_in(_f("wk", l), [128, KD, DQ])
        dram_in(_f("wv", l), [128, KD, DQ])
        dram_in(_f("wo", l), [128, DQ // 128, D])
        dram_in(_f("w1", l), [128, KD, FFC])
        dram_in(_f("w2", l), [128, FFC // 128, D])
        dram_in(_f("bqkv", l), [128, 6], F32)
        dram_in(_f("b1", l), [128, FFC // 128], F32)
        dram_in(_f("b2", l), [128, KD], F32)
    dram_in("emb", [128, KD, VSP])
    logits = nc.dram_tensor("logits", [VSP, T], BF16,
                            kind="ExternalOutput").ap()

    with tile.TileContext(nc) as tc:
        _body(tc, inp, logits)
    nc.compile()
    return nc


def _body(tc, inp, logits):
    nc = tc.nc
    ctx = contextlib.ExitStack()
    with ctx:
        # --- SBUF pools (sizes are KB/partition) ---
        singles = ctx.enter_context(tc.tile_pool(name="singles", bufs=1))
        xp = ctx.enter_context(tc.tile_pool(name="xp", bufs=1))        # 16
        npool = ctx.enter_context(tc.tile_pool(name="npool", bufs=1))  # 16
        tmp = ctx.enter_context(tc.tile_pool(name="tmp", bufs=2))      # 6
        qkv = ctx.enter_context(tc.tile_pool(name="qkv", bufs=1))      # 12
        vh = ctx.enter_context(tc.tile_pool(name="vh", bufs=2))        # 4
        et = ctx.enter_context(tc.tile_pool(name="et", bufs=2))        # 16
        op = ctx.enter_context(tc.tile_pool(name="op", bufs=1))        # 4
        hp = ctx.enter_context(tc.tile_pool(name="hp", bufs=2))        # 16
        arr = ctx.enter_context(tc.tile_pool(name="arr", bufs=1))      # 8
        wts = ctx.enter_context(tc.tile_pool(name="wts", bufs=2))      # 32
        wff = ctx.enter_context(tc.tile_pool(name="wff", bufs=1))      # 32
        embp = ctx.enter_context(tc.tile_pool(name="embp", bufs=4))    # 8
        stat = ctx.enter_context(tc.tile_pool(name="stat", bufs=2))
        bcp = ctx.enter_context(tc.tile_pool(name="bcp", bufs=1))      # 8
        lout = ctx.enter_context(tc.tile_pool(name="lout", bufs=3))
        bias = ctx.enter_context(tc.tile_pool(name="bias", bufs=3))
        stg = ctx.enter_context(tc.tile_pool(name="stg", bufs=2))      # 3
        # --- PSUM pools (8 banks total) ---
        ps_mm = ctx.enter_context(tc.tile_pool(name="ps_mm", bufs=2, space="PSUM"))
        # attention/LN PSUM pools live in attn_ctx, closed before the logit
        # phase so its banks can be reused for a deeper logit rotation
        attn_ctx = contextlib.ExitStack()
        ps_sc = attn_ctx.enter_context(
            tc.tile_pool(name="ps_sc", bufs=2, space="PSUM"))
        ps_st = attn_ctx.enter_context(
            tc.tile_pool(name="ps_st", bufs=1, space="PSUM"))
        ps_tr = attn_ctx.enter_context(
            tc.tile_pool(name="ps_tr", bufs=1, space="PSUM"))
        ps_o = attn_ctx.enter_context(
            tc.tile_pool(name="ps_o", bufs=2, space="PSUM"))
        # --- DRAM (collective bounce) ---
        dram = ctx.enter_context(tc.tile_pool(name="dram", bufs=4, space="DRAM"))

        # --- constants / persistent ---
        ones_t = singles.tile([128, 1], BF16)
        nc.sync.dma_start(out=ones_t[:], in_=inp["ones"][:])
        ident_t = singles.tile([128, 128], BF16)
        nc.sync.dma_start(out=ident_t[:], in_=inp["ident"][:])
        tri_t = singles.tile([128, 128], BF16)
        nc.sync.dma_start(out=tri_t[:], in_=inp["tri"][:])
        eps_t = singles.tile([1, 1], F32)
        nc.vector.memset(eps_t[:], 1e-5)

        xt = xp.tile([128, KD, T], BF16, tag="x")
        nc.sync.dma_start(out=xt[:], in_=inp["x0"][:])

        def layer_norm_chunk(src, dst, c):
            """dst[:,:,c] (bf16) = (src - mean) * rsqrt(var + eps)."""
            cs = ts(c, TCH)
            s12 = ps_st.tile([33, TCH], F32, tag="st")
            s1 = s12[0:1, :]
            s2 = s12[32:33, :]
            for k in range(KD):
                nc.tensor.matmul(s1[:], ones_t[:], src[:, k, cs],
                                 start=(k == 0), stop=(k == KD - 1))
            for k in range(KD):
                sq = tmp.tile([128, TCH], BF16, tag="tmp")
                nc.vector.tensor_tensor(
                    out=sq[:], in0=src[:, k, cs], in1=src[:, k, cs],
                    op=ALU.mult)
                nc.tensor.matmul(s2[:], ones_t[:], sq[:],
                                 start=(k == 0), stop=(k == KD - 1))
            # finishing math on [1, TCH] rows
            m = stat.tile([1, TCH], F32, tag="sa")
            nc.vector.tensor_scalar_mul(m[:], s1[:], 1.0 / D)
            msq = stat.tile([1, TCH], F32, tag="sb")
            nc.vector.tensor_tensor(out=msq[:], in0=m[:], in1=m[:],
                                    op=ALU.mult)
            var = stat.tile([1, TCH], F32, tag="sb")
            nc.vector.scalar_tensor_tensor(
                out=var[:], in0=s2[:], scalar=1.0 / D, in1=msq[:],
                op0=ALU.mult, op1=ALU.subtract)
            sd = stat.tile([1, TCH], F32, tag="sb")
            nc.scalar.activation(sd[:], var[:], AF.Sqrt, bias=eps_t[:])
            rs = stat.tile([1, TCH], F32, tag="sb")
            nc.vector.reciprocal_approx_fast(rs[:], sd[:])
            rB = bcp.tile([128, TCH], F32, tag="rB")
            nc.gpsimd.partition_broadcast(rB[:], rs[:])
            mB = bcp.tile([128, TCH], F32, tag="mB")
            nc.gpsimd.partition_broadcast(mB[:], m[:])
            # dst = (src - mB) * rB   (two DVE passes per k)
            for k in range(KD):
                t2 = tmp.tile([128, TCH], F32, tag="tmp2")
                nc.vector.tensor_tensor(
                    out=t2[:], in0=src[:, k, cs], in1=mB[:], op=ALU.subtract)
                nc.vector.tensor_tensor(
                    out=dst[:, k, cs], in0=t2[:], in1=rB[:], op=ALU.mult)

        def proj_chunk(dst, wt, n_src, mchunks, bias_t, bcol0, c):
            """dst[:, m, c-chunk] (bf16) = W^T @ n_src[c-chunk] + bias."""
            cs = ts(c, TCH)
            for m in range(mchunks):
                pt = ps_mm.tile([128, TCH], F32, tag="mm")
                for k in range(KD):
                    nc.tensor.matmul(pt[:], wt[:, k, ts(m, 128)],
                                     n_src[:, k, cs],
                                     start=(k == 0), stop=(k == KD - 1))
                nc.scalar.activation(
                    dst[:, m, cs], pt[:], AF.Identity,
                    bias=bias_t[:, bcol0 + m:bcol0 + m + 1])

        def qkv_head_chunk(c, n_src, qT, kT, vT, wq_t, wk_t, wv_t, bqkv_t,
                           vhs2):
            """QKV projections for token chunk c, plus the v-transposes
            whose key blocks live in chunk c. Each [128,128] transpose
            covers both heads of a head-pair; the transposed values land
            in vhs2[mh] = [128, KD, 130] with an all-ones column at 64
            and 129 so the AV matmul also produces the softmax row sums
            (output row 64 of a 65-row result)."""
            proj_chunk(qT, wq_t, n_src, DQ // 128, bqkv_t, 0, c)
            proj_chunk(kT, wk_t, n_src, DQ // 128, bqkv_t, 2, c)
            proj_chunk(vT, wv_t, n_src, DQ // 128, bqkv_t, 4, c)
            for mh in range(DQ // 128):
                for j in range(4 * c, 4 * c + 4):
                    tpv = ps_tr.tile([128, 128], BF16, tag="tr")
                    nc.tensor.matmul(tpv[:], vT[:, mh, ts(j, 128)],
                                     ident_t[:], is_transpose=True,
                                     start=True, stop=True)
                    nc.scalar.copy(vhs2[mh][:, j, 0:64], tpv[:, 0:64])
                    nc.vector.tensor_scalar_add(vhs2[mh][:, j, 65:129],
                                                tpv[:, 64:128], 0.0)

        def attn_chunk(l, c, qT, kT, vhs2, oT, inject=None, inject_at=1):
            """Scores are computed pre-transposed: eT[k, q] = exp(qk^T+mask)
            written straight to SBUF by the Exp activation (k-tile is the
            stationary operand), so no per-block transposes of the probs
            are needed. The AV matmul contracts over keys and its 65th
            output row (ones column of vhs2) is the softmax denominator;
            o is normalized by a per-token reciprocal on the way out."""
            nkj = 4 * (c + 1)
            for h in range(HPC):
                if h == inject_at and inject is not None:
                    inject()
                pp = 64 * (h % 2)
                mh = h // 2
                hh = h % 2
                etile = et.tile([128, KD, TCH], BF16, tag="et")
                for kj in range(nkj):
                    q0 = max(0, 128 * kj - TCH * c)
                    diag = 128 * kj - TCH * c >= 0
                    spt = ps_sc.tile([128, 512], F32, tag="sc")
                    nc.tensor.matmul(
                        spt[:, q0:TCH],
                        kT[pp:pp + 64, mh, ts(kj, 128)],
                        qT[pp:pp + 64, mh, TCH * c + q0:TCH * (c + 1)],
                        start=True, stop=not diag)
                    if diag:
                        # causal mask for the diagonal block (tri is
                        # strictly-lower-triangular MASK_VAL in [k, q])
                        nc.tensor.matmul(spt[:, q0:q0 + 128], ident_t[:],
                                         tri_t[:], start=False, stop=True)
                    nc.scalar.activation(etile[:, kj, q0:TCH],
                                         spt[:, q0:TCH], AF.Exp)
                po = ps_o.tile([65, TCH], F32, tag="o")
                for kj in range(nkj):
                    lo = max(0, 128 * kj - TCH * c)
                    nc.tensor.matmul(po[:, lo:TCH],
                                     vhs2[mh][:, kj, 65 * hh:65 * hh + 65],
                                     etile[:, kj, lo:TCH],
                                     start=(kj == 0), stop=(kj == nkj - 1))
                rsS = stat.tile([1, TCH], F32, tag="rsS")
                nc.vector.tensor_scalar_add(rsS[:], po[64:65, :], 0.0)
                rs = stat.tile([1, TCH], F32, tag="rs")
                nc.vector.reciprocal_approx_fast(rs[:], rsS[:])
                rB2 = bcp.tile([64, TCH], F32, tag="rb2")
                nc.gpsimd.partition_broadcast(rB2[:], rs[:])
                nc.vector.tensor_tensor(
                    out=oT[pp:pp + 64, mh, ts(c, TCH)],
                    in0=po[0:64, :], in1=rB2[:], op=ALU.mult)

        def mm_ar_chunk(wt, kchunks, src_tile, c, src_is_chunk):
            """out-partial = W^T @ src for chunk c -> bf16 AllReduce."""
            ar_in = dram.tile([128, KD, TCH], BF16, tag="dr")
            ar_out = dram.tile([128, KD, TCH], BF16, tag="dr")
            for m in range(KD):
                pt = ps_mm.tile([128, TCH], F32, tag="mm")
                for k in range(kchunks):
                    s = (src_tile[:, k, :] if src_is_chunk
                         else src_tile[:, k, ts(c, TCH)])
                    nc.tensor.matmul(pt[:], wt[:, k, ts(m, 128)], s,
                                     start=(k == 0), stop=(k == kchunks - 1))
                st_ = stg.tile([128, TCH], BF16, tag="stg")
                nc.scalar.copy(st_[:], pt[:])
                nc.sync.dma_start(out=ar_in[:, m, :], in_=st_[:])
            nc.gpsimd.collective_compute(
                "AllReduce", ALU.add, replica_groups=REPLICA_GROUPS,
                ins=[ar_in.opt()], outs=[ar_out.opt()])
            return ar_out

        def load_w(name, kd, width, pool_tag, bufs_pool):
            wt = bufs_pool.tile([128, kd, width], BF16, tag=pool_tag,
                                name=f"{name}_t")
            nc.sync.dma_start(out=wt[:], in_=inp[name][:])
            return wt

        def make_vhs2(l):
            vhs2 = []
            for mh in range(DQ // 128):
                v2 = vh.tile([128, KD, 130], BF16, tag="vh",
                             name=f"vh{l}_{mh}")
                nc.vector.memset(v2[:, :, 64:65], 1.0)
                nc.vector.memset(v2[:, :, 129:130], 1.0)
                vhs2.append(v2)
            return vhs2

        # ---------------- prologue: LN1 + QKV of layer 0 ----------------
        n_cur = npool.tile([128, KD, T], BF16, tag="n")
        bqkv_t = bias.tile([128, 6], F32, tag="bias")
        nc.sync.dma_start(out=bqkv_t[:], in_=inp[_f("bqkv", 0)][:])
        wq_t = load_w(_f("wq", 0), KD, DQ, "wq", wts)
        wk_t = load_w(_f("wk", 0), KD, DQ, "wk", wts)
        wv_t = load_w(_f("wv", 0), KD, DQ, "wv", wts)
        wo_t = load_w(_f("wo", 0), DQ // 128, D, "wo", wts)
        qT = qkv.tile([128, DQ // 128, T], BF16, tag="qT")
        kT = qkv.tile([128, DQ // 128, T], BF16, tag="kT")
        vT = qkv.tile([128, DQ // 128, T], BF16, tag="vT")
        vhs2 = make_vhs2(0)
        for c in range(NTC):
            layer_norm_chunk(xt, n_cur, c)
            qkv_head_chunk(c, n_cur, qT, kT, vT, wq_t, wk_t, wv_t, bqkv_t,
                           vhs2)

        def ffn_w1(l, c, n2, w1_t, b1_t):
            cs = ts(c, TCH)
            hT = hp.tile([128, FFC // 128, TCH], BF16, tag="h",
                         name=f"hT{l}_{c}")
            for m in range(FFC // 128):
                pt = ps_mm.tile([128, TCH], F32, tag="mm", name=f"p1_{l}_{c}_{m}")
                for k in range(KD):
                    nc.tensor.matmul(pt[:], w1_t[:, k, ts(m, 128)],
                                     n2[:, k, cs],
                                     start=(k == 0), stop=(k == KD - 1))
                nc.scalar.activation(hT[:, m, :], pt[:], AF.Gelu,
                                     bias=b1_t[:, m:m + 1])
            return hT

        def add_f_lnnext(c, ar_f, b2_t, n_next):
            cs = ts(c, TCH)
            art = arr.tile([128, KD, TCH], BF16, tag="arr", name=f"artf{c}")
            nc.sync.dma_start(out=art[:], in_=ar_f[c][:])
            for m in range(KD):
                nc.vector.scalar_tensor_tensor(
                    out=xt[:, m, cs], in0=art[:, m, :],
                    scalar=b2_t[:, m:m + 1],
                    in1=xt[:, m, cs],
                    op0=ALU.add, op1=ALU.add)
            layer_norm_chunk(xt, n_next, c)

        for l in range(L):
            # ---------------- attention + Wo + AR, chunk-pipelined ----------------
            w1_t = load_w(_f("w1", l), KD, FFC, "w1", wff)
            w2_t = load_w(_f("w2", l), FFC // 128, D, "w2", wff)
            oT = op.tile([128, DQ // 128, T], BF16, tag="oT")
            n2 = npool.tile([128, KD, T], BF16, tag="n")
            ar_a = []

            def add_a(c):
                art = arr.tile([128, KD, TCH], BF16, tag="arr",
                               name=f"arta{c}_{l}")
                nc.sync.dma_start(out=art[:], in_=ar_a[c][:])
                for k in range(KD):
                    nc.vector.tensor_tensor(
                        out=xt[:, k, ts(c, TCH)],
                        in0=xt[:, k, ts(c, TCH)],
                        in1=art[:, k, :], op=ALU.add)

            def add_a_ln2_c0():
                add_a(0)
                layer_norm_chunk(xt, n2, 0)

            attn_chunk(l, 0, qT, kT, vhs2, oT)
            ar_a.append(mm_ar_chunk(wo_t, DQ // 128, oT, 0, False))
            attn_chunk(l, 1, qT, kT, vhs2, oT, inject=add_a_ln2_c0)
            ar_a.append(mm_ar_chunk(wo_t, DQ // 128, oT, 1, False))

            # ------- residual c1 + LN2(c1) overlap FFN(c0) -------
            b1_t = bias.tile([128, FFC // 128], F32, tag="bias")
            nc.sync.dma_start(out=b1_t[:], in_=inp[_f("b1", l)][:])
            b2_t = bias.tile([128, KD], F32, tag="bias")
            nc.sync.dma_start(out=b2_t[:], in_=inp[_f("b2", l)][:])
            add_a(1)
            layer_norm_chunk(xt, n2, 1)

            ar_f = []
            hT0 = ffn_w1(l, 0, n2, w1_t, b1_t)
            ar_f.append(mm_ar_chunk(w2_t, FFC // 128, hT0, 0, True))
            hT1 = ffn_w1(l, 1, n2, w1_t, b1_t)

            n_next = npool.tile([128, KD, T], BF16, tag="n")
            ar_f.append(mm_ar_chunk(w2_t, FFC // 128, hT1, 1, True))
            add_f_lnnext(0, ar_f, b2_t, n_next)

            if l < L - 1:
                bqkv_t = bias.tile([128, 6], F32, tag="bias")
                nc.sync.dma_start(out=bqkv_t[:], in_=inp[_f("bqkv", l + 1)][:])
                wq_t = load_w(_f("wq", l + 1), KD, DQ, "wq", wts)
                wk_t = load_w(_f("wk", l + 1), KD, DQ, "wk", wts)
                wv_t = load_w(_f("wv", l + 1), KD, DQ, "wv", wts)
                wo_t = load_w(_f("wo", l + 1), DQ // 128, D, "wo", wts)
                qT = qkv.tile([128, DQ // 128, T], BF16, tag="qT")
                kT = qkv.tile([128, DQ // 128, T], BF16, tag="kT")
                vT = qkv.tile([128, DQ // 128, T], BF16, tag="vT")
                vhs2 = make_vhs2(l + 1)
                qkv_head_chunk(0, n_next, qT, kT, vT, wq_t, wk_t, wv_t,
                               bqkv_t, vhs2)
            add_f_lnnext(1, ar_f, b2_t, n_next)
            if l < L - 1:
                qkv_head_chunk(1, n_next, qT, kT, vT, wq_t, wk_t, wv_t,
                               bqkv_t, vhs2)
            n_cur = n_next

        # after the loop, n_cur holds the final layer norm
        nf = n_cur
        attn_ctx.close()
        ps_lg = ctx.enter_context(
            tc.tile_pool(name="ps_lg", bufs=4, space="PSUM"))

        # logits^T [VSP, T]: emb strip is the stationary operand, nf moves.
        # A t2=0-only prefix runs first: it depends only on chunk 0 of the
        # final layernorm, hiding the last AllReduce + LN tail of chunk 1.
        NPRE = 16

        def logit_block(vb, t2, ebt):
            pt = ps_lg.tile([128, TCH], F32, tag="lg",
                            name=f"plg{vb}_{t2}")
            for k in range(KD):
                nc.tensor.matmul(pt[:], ebt[:, k, :],
                                 nf[:, k, ts(t2, TCH)],
                                 start=(k == 0), stop=(k == KD - 1))
            lo = lout.tile([128, TCH], BF16, tag="lo", name=f"lo{vb}_{t2}")
            nc.scalar.copy(lo[:], pt[:])
            nc.sync.dma_start(out=logits[ts(vb, 128), ts(t2, TCH)],
                              in_=lo[:])

        for vb in range(NPRE):
            ebt = embp.tile([128, KD, 128], BF16, tag="emb", name=f"ebA{vb}")
            nc.sync.dma_start(out=ebt[:], in_=inp["emb"][:, :, ts(vb, 128)])
            logit_block(vb, 0, ebt)
        for vb in range(VSP // 128):
            ebt = embp.tile([128, KD, 128], BF16, tag="emb", name=f"ebB{vb}")
            nc.sync.dma_start(out=ebt[:], in_=inp["emb"][:, :, ts(vb, 128)])
            for t2 in range(NTC):
                if t2 == 0 and vb < NPRE:
                    continue
                logit_block(vb, t2, ebt)


# ------------------------------------------------------------------
# Host side
# ------------------------------------------------------------------

def _bf(a):
    import ml_dtypes

    return np.asarray(a, np.float32).astype(ml_dtypes.bfloat16)


def _kfold(w):
    """[in, out] -> [128, in//128, out] K-tiled layout."""
    i, o = w.shape
    return np.ascontiguousarray(
        w.reshape(i // 128, 128, o).transpose(1, 0, 2))


def _cols(v):
    """[n] -> [128, n//128] per-partition bias columns."""
    return np.ascontiguousarray(v.reshape(-1, 128).T)


def prep_inputs(inputs):
    """Full inputs -> list of 8 per-core input maps."""
    f = lambda a: np.asarray(a, np.float32)
    tokens = np.asarray(inputs["tokens"])
    tok_emb, pos_emb = f(inputs["tok_emb"]), f(inputs["pos_emb"])
    ln1_g, ln1_b = f(inputs["ln1_g"]), f(inputs["ln1_b"])
    wq, wk = f(inputs["wq"]), f(inputs["wk"])
    wv, wo = f(inputs["wv"]), f(inputs["wo"])
    ln2_g, ln2_b = f(inputs["ln2_g"]), f(inputs["ln2_b"])
    w1, b1 = f(inputs["w1"]), f(inputs["b1"])
    w2, b2 = f(inputs["w2"]), f(inputs["b2"])
    lnf_g = f(inputs["lnf_g"])

    sc = 1.0 / np.sqrt(HD)
    x0 = tok_emb[tokens] + pos_emb[:S][None]          # [B, S, D]
    ones = np.ones((128, 1), np.float32)
    ident = np.eye(128, dtype=np.float32)
    # strictly-lower-triangular mask in [key, query] layout
    tri = np.tril(np.full((128, 128), MASK_VAL, np.float32), k=-1)

    in_maps = []
    for core in range(N_CORES):
        b = core // TP
        tpr = core % TP
        m = {
            "x0": _bf(_kfold(np.ascontiguousarray(x0[b].T))),
            "ones": _bf(ones), "ident": _bf(ident), "tri": _bf(tri),
        }
        qs = slice(tpr * DQ, (tpr + 1) * DQ)
        fs = slice(tpr * FFC, (tpr + 1) * FFC)
        for l in range(L):
            wql = wq[l][:, qs] * sc
            wkl = wk[l][:, qs]
            wvl = wv[l][:, qs]
            m[_f("wq", l)] = _bf(_kfold(ln1_g[l][:, None] * wql))
            m[_f("wk", l)] = _bf(_kfold(ln1_g[l][:, None] * wkl))
            m[_f("wv", l)] = _bf(_kfold(ln1_g[l][:, None] * wvl))
            m[_f("wo", l)] = _bf(_kfold(wo[l][qs, :]))
            m[_f("w1", l)] = _bf(_kfold(ln2_g[l][:, None] * w1[l][:, fs]))
            m[_f("w2", l)] = _bf(_kfold(w2[l][fs, :]))
            m[_f("bqkv", l)] = np.concatenate(
                [_cols(ln1_b[l] @ wql), _cols(ln1_b[l] @ wkl),
                 _cols(ln1_b[l] @ wvl)], axis=1).astype(np.float32)
            m[_f("b1", l)] = _cols(b1[l][fs] + ln2_b[l] @ w1[l][:, fs]).astype(
                np.float32)
            m[_f("b2", l)] = _cols(b2[l]).astype(np.float32)
        v0 = tpr * VS
        v1 = min(v0 + VS, V)
        epad = np.zeros((D, VSP), np.float32)
        epad[:, :v1 - v0] = (tok_emb[v0:v1] * lnf_g[None, :]).T
        m["emb"] = _bf(_kfold(epad))
        in_maps.append(m)
    return in_maps


_CACHED = {}


def _get_program():
    if "nc" not in _CACHED:
        _CACHED["nc"] = build_program()
    return _CACHED["nc"]


def run(inputs, trace=False, **kw):
    nc = _get_program()
    in_maps = prep_inputs(inputs)
    return run_bass_kernel_spmd(nc, in_maps, list(range(N_CORES)),
                                trace=trace, **kw)


def assemble(results, inputs):
    """Per-core logits -> full [B, S, V] float32."""
    lnf_b = np.asarray(inputs["lnf_b"], np.float32)
    tok_emb = np.asarray(inputs["tok_emb"], np.float32)
    out = np.empty((B, S, V), np.float32)
    for b in range(B):
        parts = []
        for tpr in range(TP):
            v0 = tpr * VS
            v1 = min(v0 + VS, V)
            parts.append(
                results[b * TP + tpr]["logits"][:v1 - v0, :].T.astype(
                    np.float32))
        out[b] = np.concatenate(parts, axis=1)
    if np.any(lnf_b):
        out += (tok_emb @ lnf_b)[None, None, :]
    return out


def kernel(**inputs):
    res = run(inputs)
    return assemble(res.results, inputs)


if __name__ == "__main__":
    print("building program...")
    build_program()
    print("build + compile OK")


# revision 46
# speedup vs baseline: 2.1459x; 1.0087x over previous
"""GPT forward pass on 8 Trainium2 NeuronCores (Bass/Tile), bf16 compute.

Model: B=2, S=1024, D=1024, H=16 heads (hd=64), L=6 layers, V=50257,
tied embedding head.

Sharding: DP2 x TP4. Cores 0-3 compute batch element 0, cores 4-7
batch element 1. Within each group of 4: attention is head-sharded
(4 heads/core), the FFN hidden dim is column/row sharded (1024/core),
and the tied logit matrix is vocab-sharded (12565 rows/core, padded
to 12800). Two AllReduces per layer (post-attention, post-FFN) over
each group of 4, token-chunked (2 x 512) so collectives overlap
compute.

On-device layout: activations are feature-major ("transposed"):
x[p, k, t] = X^T[128k + p, t]. All matmuls take weights [in, out] as
the stationary operand and activations [in, tokens] as the moving
operand, producing outputs feature-major with zero activation
transposes. LayerNorm gains/biases and the attention 1/sqrt(hd) scale
are folded into the adjacent weights on the host, so the device only
computes the pure normalization (x - mean) * rsqrt(var + eps), with
stats via ones-matmuls on the PE and per-token broadcasts via GpSimd.

All matmul datapaths are bf16 (weights cast on host, activations cast
on write); PSUM accumulation stays fp32, the residual stream stays
fp32 in SBUF, and the AllReduce payload is bf16.
"""

import sys

sys.path.insert(0, "/opt/trn_rl_repo")

import contextlib

import numpy as np

import concourse.bacc as bacc
import concourse.mybir as mybir
import concourse.tile as tile
from concourse.bass import ts
from concourse.bass_utils import run_bass_kernel_spmd

F32 = mybir.dt.float32
BF16 = mybir.dt.bfloat16
AF = mybir.ActivationFunctionType
ALU = mybir.AluOpType

# Model dims
B, S, D, H, L, V = 2, 1024, 1024, 16, 6, 50257
HD = D // H           # 64
DFF = 4 * D           # 4096
N_CORES = 8
TP = 4                # tensor-parallel group size
HPC = H // TP         # heads per core = 4
DQ = HPC * HD         # per-core qkv width = 256
FFC = DFF // TP       # per-core ffn hidden = 1024
KD = D // 128         # 8 feature tiles
T = S                 # tokens per core (one batch element)
TCH = 512             # token chunk for AR pipelining
NTC = T // TCH        # 2
VS = 12565            # vocab rows per core (last core: 12562)
VSP = 12800           # padded vocab rows per core
MASK_VAL = -60.0

REPLICA_GROUPS = [[0, 1, 2, 3], [4, 5, 6, 7]]


def _f(name, l=None):
    return name if l is None else f"{name}{l}"


def build_program():
    """Build the SPMD bass program (same instruction stream on all cores)."""
    nc = bacc.Bacc("TRN2", target_bir_lowering=False, debug=False,
                   enable_asserts=True, num_devices=N_CORES)

    inp = {}

    def dram_in(name, shape, dtype=BF16):
        inp[name] = nc.dram_tensor(name, shape, dtype, kind="ExternalInput").ap()
        return inp[name]

    dram_in("x0", [128, KD, T])
    dram_in("ones", [128, 1])
    dram_in("ident", [128, 128])
    dram_in("tri", [128, 128])
    for l in range(L):
        dram_in(_f("wq", l), [128, KD, DQ])
        dram_in(_f("wk", l), [128, KD, DQ])
        dram_in(_f("wv", l), [128, KD, DQ])
        dram_in(_f("wo", l), [128, DQ // 128, D])
        dram_in(_f("w1", l), [128, KD, FFC])
        dram_in(_f("w2", l), [128, FFC // 128, D])
        dram_in(_f("bqkv", l), [128, 6], F32)
        dram_in(_f("b1", l), [128, FFC // 128], F32)
        dram_in(_f("b2", l), [128, KD], F32)
    dram_in("emb", [128, KD, VSP])
    logits = nc.dram_tensor("logits", [VSP, T], BF16,
                            kind="ExternalOutput").ap()

    with tile.TileContext(nc) as tc:
        _body(tc, inp, logits)
    nc.compile()
    return nc


def _body(tc, inp, logits):
    nc = tc.nc
    ctx = contextlib.ExitStack()
    with ctx:
        # --- SBUF pools (sizes are KB/partition) ---
        singles = ctx.enter_context(tc.tile_pool(name="singles", bufs=1))
        xp = ctx.enter_context(tc.tile_pool(name="xp", bufs=1))        # 16
        npool = ctx.enter_context(tc.tile_pool(name="npool", bufs=1))  # 16
        tmp = ctx.enter_context(tc.tile_pool(name="tmp", bufs=2))      # 6
        qkv = ctx.enter_context(tc.tile_pool(name="qkv", bufs=1))      # 12
        vh = ctx.enter_context(tc.tile_pool(name="vh", bufs=2))        # 4
        et = ctx.enter_context(tc.tile_pool(name="et", bufs=2))        # 16
        op = ctx.enter_context(tc.tile_pool(name="op", bufs=1))        # 4
        hp = ctx.enter_context(tc.tile_pool(name="hp", bufs=2))        # 16
        arr = ctx.enter_context(tc.tile_pool(name="arr", bufs=1))      # 8
        wts = ctx.enter_context(tc.tile_pool(name="wts", bufs=2))      # 32
        wff = ctx.enter_context(tc.tile_pool(name="wff", bufs=1))      # 32
        embp = ctx.enter_context(tc.tile_pool(name="embp", bufs=6))    # 12
        stat = ctx.enter_context(tc.tile_pool(name="stat", bufs=2))
        bcp = ctx.enter_context(tc.tile_pool(name="bcp", bufs=1))      # 8
        lout = ctx.enter_context(tc.tile_pool(name="lout", bufs=3))
        bias = ctx.enter_context(tc.tile_pool(name="bias", bufs=3))
        stg = ctx.enter_context(tc.tile_pool(name="stg", bufs=2))      # 3
        # --- PSUM pools (8 banks total) ---
        ps_mm = ctx.enter_context(tc.tile_pool(name="ps_mm", bufs=2, space="PSUM"))
        # attention/LN PSUM pools live in attn_ctx, closed before the logit
        # phase so its banks can be reused for a deeper logit rotation
        attn_ctx = contextlib.ExitStack()
        ps_sc = attn_ctx.enter_context(
            tc.tile_pool(name="ps_sc", bufs=2, space="PSUM"))
        ps_st = attn_ctx.enter_context(
            tc.tile_pool(name="ps_st", bufs=1, space="PSUM"))
        ps_tr = attn_ctx.enter_context(
            tc.tile_pool(name="ps_tr", bufs=1, space="PSUM"))
        ps_o = attn_ctx.enter_context(
            tc.tile_pool(name="ps_o", bufs=2, space="PSUM"))
        # --- DRAM (collective bounce) ---
        dram = ctx.enter_context(tc.tile_pool(name="dram", bufs=4, space="DRAM"))

        # --- constants / persistent ---
        ones_t = singles.tile([128, 1], BF16)
        nc.sync.dma_start(out=ones_t[:], in_=inp["ones"][:])
        ident_t = singles.tile([128, 128], BF16)
        nc.sync.dma_start(out=ident_t[:], in_=inp["ident"][:])
        tri_t = singles.tile([128, 128], BF16)
        nc.sync.dma_start(out=tri_t[:], in_=inp["tri"][:])
        eps_t = singles.tile([1, 1], F32)
        nc.vector.memset(eps_t[:], 1e-5)

        xt = xp.tile([128, KD, T], BF16, tag="x")
        nc.sync.dma_start(out=xt[:], in_=inp["x0"][:])

        def layer_norm_chunk(src, dst, c):
            """dst[:,:,c] (bf16) = (src - mean) * rsqrt(var + eps)."""
            cs = ts(c, TCH)
            s12 = ps_st.tile([33, TCH], F32, tag="st")
            s1 = s12[0:1, :]
            s2 = s12[32:33, :]
            for k in range(KD):
                nc.tensor.matmul(s1[:], ones_t[:], src[:, k, cs],
                                 start=(k == 0), stop=(k == KD - 1))
            for k in range(KD):
                sq = tmp.tile([128, TCH], BF16, tag="tmp")
                nc.vector.tensor_tensor(
                    out=sq[:], in0=src[:, k, cs], in1=src[:, k, cs],
                    op=ALU.mult)
                nc.tensor.matmul(s2[:], ones_t[:], sq[:],
                                 start=(k == 0), stop=(k == KD - 1))
            # finishing math on [1, TCH] rows
            m = stat.tile([1, TCH], F32, tag="sa")
            nc.vector.tensor_scalar_mul(m[:], s1[:], 1.0 / D)
            msq = stat.tile([1, TCH], F32, tag="sb")
            nc.vector.tensor_tensor(out=msq[:], in0=m[:], in1=m[:],
                                    op=ALU.mult)
            var = stat.tile([1, TCH], F32, tag="sb")
            nc.vector.scalar_tensor_tensor(
                out=var[:], in0=s2[:], scalar=1.0 / D, in1=msq[:],
                op0=ALU.mult, op1=ALU.subtract)
            sd = stat.tile([1, TCH], F32, tag="sb")
            nc.scalar.activation(sd[:], var[:], AF.Sqrt, bias=eps_t[:])
            rs = stat.tile([1, TCH], F32, tag="sb")
            nc.vector.reciprocal_approx_fast(rs[:], sd[:])
            rB = bcp.tile([128, TCH], F32, tag="rB")
            nc.gpsimd.partition_broadcast(rB[:], rs[:])
            mB = bcp.tile([128, TCH], F32, tag="mB")
            nc.gpsimd.partition_broadcast(mB[:], m[:])
            # dst = (src - mB) * rB   (two DVE passes per k)
            for k in range(KD):
                t2 = tmp.tile([128, TCH], F32, tag="tmp2")
                nc.vector.tensor_tensor(
                    out=t2[:], in0=src[:, k, cs], in1=mB[:], op=ALU.subtract)
                nc.vector.tensor_tensor(
                    out=dst[:, k, cs], in0=t2[:], in1=rB[:], op=ALU.mult)

        def proj_chunk(dst, wt, n_src, mchunks, bias_t, bcol0, c):
            """dst[:, m, c-chunk] (bf16) = W^T @ n_src[c-chunk] + bias."""
            cs = ts(c, TCH)
            for m in range(mchunks):
                pt = ps_mm.tile([128, TCH], F32, tag="mm")
                for k in range(KD):
                    nc.tensor.matmul(pt[:], wt[:, k, ts(m, 128)],
                                     n_src[:, k, cs],
                                     start=(k == 0), stop=(k == KD - 1))
                nc.scalar.activation(
                    dst[:, m, cs], pt[:], AF.Identity,
                    bias=bias_t[:, bcol0 + m:bcol0 + m + 1])

        def qkv_head_chunk(c, n_src, qT, kT, vT, wq_t, wk_t, wv_t, bqkv_t,
                           vhs2):
            """QKV projections for token chunk c, plus the v-transposes
            whose key blocks live in chunk c. Each [128,128] transpose
            covers both heads of a head-pair; the transposed values land
            in vhs2[mh] = [128, KD, 130] with an all-ones column at 64
            and 129 so the AV matmul also produces the softmax row sums
            (output row 64 of a 65-row result)."""
            proj_chunk(qT, wq_t, n_src, DQ // 128, bqkv_t, 0, c)
            proj_chunk(kT, wk_t, n_src, DQ // 128, bqkv_t, 2, c)
            proj_chunk(vT, wv_t, n_src, DQ // 128, bqkv_t, 4, c)
            for mh in range(DQ // 128):
                for j in range(4 * c, 4 * c + 4):
                    tpv = ps_tr.tile([128, 128], BF16, tag="tr")
                    nc.tensor.matmul(tpv[:], vT[:, mh, ts(j, 128)],
                                     ident_t[:], is_transpose=True,
                                     start=True, stop=True)
                    nc.scalar.copy(vhs2[mh][:, j, 0:64], tpv[:, 0:64])
                    nc.vector.tensor_scalar_add(vhs2[mh][:, j, 65:129],
                                                tpv[:, 64:128], 0.0)

        def attn_chunk(l, c, qT, kT, vhs2, oT, inject=None, inject_at=1):
            """Scores are computed pre-transposed: eT[k, q] = exp(qk^T+mask)
            written straight to SBUF by the Exp activation (k-tile is the
            stationary operand), so no per-block transposes of the probs
            are needed. The AV matmul contracts over keys and its 65th
            output row (ones column of vhs2) is the softmax denominator;
            o is normalized by a per-token reciprocal on the way out."""
            nkj = 4 * (c + 1)
            for h in range(HPC):
                if h == inject_at and inject is not None:
                    inject()
                pp = 64 * (h % 2)
                mh = h // 2
                hh = h % 2
                etile = et.tile([128, KD, TCH], BF16, tag="et")
                for kj in range(nkj):
                    q0 = max(0, 128 * kj - TCH * c)
                    diag = 128 * kj - TCH * c >= 0
                    spt = ps_sc.tile([128, 512], F32, tag="sc")
                    nc.tensor.matmul(
                        spt[:, q0:TCH],
                        kT[pp:pp + 64, mh, ts(kj, 128)],
                        qT[pp:pp + 64, mh, TCH * c + q0:TCH * (c + 1)],
                        start=True, stop=not diag)
                    if diag:
                        # causal mask for the diagonal block (tri is
                        # strictly-lower-triangular MASK_VAL in [k, q])
                        nc.tensor.matmul(spt[:, q0:q0 + 128], ident_t[:],
                                         tri_t[:], start=False, stop=True)
                    nc.scalar.activation(etile[:, kj, q0:TCH],
                                         spt[:, q0:TCH], AF.Exp)
                po = ps_o.tile([65, TCH], F32, tag="o")
                for kj in range(nkj):
                    lo = max(0, 128 * kj - TCH * c)
                    nc.tensor.matmul(po[:, lo:TCH],
                                     vhs2[mh][:, kj, 65 * hh:65 * hh + 65],
                                     etile[:, kj, lo:TCH],
                                     start=(kj == 0), stop=(kj == nkj - 1))
                rsS = stat.tile([1, TCH], F32, tag="rsS")
                nc.vector.tensor_scalar_add(rsS[:], po[64:65, :], 0.0)
                rs = stat.tile([1, TCH], F32, tag="rs")
                nc.vector.reciprocal_approx_fast(rs[:], rsS[:])
                rB2 = bcp.tile([64, TCH], F32, tag="rb2")
                nc.gpsimd.partition_broadcast(rB2[:], rs[:])
                nc.vector.tensor_tensor(
                    out=oT[pp:pp + 64, mh, ts(c, TCH)],
                    in0=po[0:64, :], in1=rB2[:], op=ALU.mult)

        def mm_ar_chunk(wt, kchunks, src_tile, c, src_is_chunk):
            """out-partial = W^T @ src for chunk c -> bf16 AllReduce."""
            ar_in = dram.tile([128, KD, TCH], BF16, tag="dr")
            ar_out = dram.tile([128, KD, TCH], BF16, tag="dr")
            for m in range(KD):
                pt = ps_mm.tile([128, TCH], F32, tag="mm")
                for k in range(kchunks):
                    s = (src_tile[:, k, :] if src_is_chunk
                         else src_tile[:, k, ts(c, TCH)])
                    nc.tensor.matmul(pt[:], wt[:, k, ts(m, 128)], s,
                                     start=(k == 0), stop=(k == kchunks - 1))
                st_ = stg.tile([128, TCH], BF16, tag="stg")
                nc.scalar.copy(st_[:], pt[:])
                nc.sync.dma_start(out=ar_in[:, m, :], in_=st_[:])
            nc.gpsimd.collective_compute(
                "AllReduce", ALU.add, replica_groups=REPLICA_GROUPS,
                ins=[ar_in.opt()], outs=[ar_out.opt()])
            return ar_out

        def load_w(name, kd, width, pool_tag, bufs_pool):
            wt = bufs_pool.tile([128, kd, width], BF16, tag=pool_tag,
                                name=f"{name}_t")
            nc.sync.dma_start(out=wt[:], in_=inp[name][:])
            return wt

        def make_vhs2(l):
            vhs2 = []
            for mh in range(DQ // 128):
                v2 = vh.tile([128, KD, 130], BF16, tag="vh",
                             name=f"vh{l}_{mh}")
                nc.vector.memset(v2[:, :, 64:65], 1.0)
                nc.vector.memset(v2[:, :, 129:130], 1.0)
                vhs2.append(v2)
            return vhs2

        # ---------------- prologue: LN1 + QKV of layer 0 ----------------
        n_cur = npool.tile([128, KD, T], BF16, tag="n")
        bqkv_t = bias.tile([128, 6], F32, tag="bias")
        nc.sync.dma_start(out=bqkv_t[:], in_=inp[_f("bqkv", 0)][:])
        wq_t = load_w(_f("wq", 0), KD, DQ, "wq", wts)
        wk_t = load_w(_f("wk", 0), KD, DQ, "wk", wts)
        wv_t = load_w(_f("wv", 0), KD, DQ, "wv", wts)
        wo_t = load_w(_f("wo", 0), DQ // 128, D, "wo", wts)
        qT = qkv.tile([128, DQ // 128, T], BF16, tag="qT")
        kT = qkv.tile([128, DQ // 128, T], BF16, tag="kT")
        vT = qkv.tile([128, DQ // 128, T], BF16, tag="vT")
        vhs2 = make_vhs2(0)
        for c in range(NTC):
            layer_norm_chunk(xt, n_cur, c)
            qkv_head_chunk(c, n_cur, qT, kT, vT, wq_t, wk_t, wv_t, bqkv_t,
                           vhs2)

        def ffn_w1(l, c, n2, w1_t, b1_t):
            cs = ts(c, TCH)
            hT = hp.tile([128, FFC // 128, TCH], BF16, tag="h",
                         name=f"hT{l}_{c}")
            for m in range(FFC // 128):
                pt = ps_mm.tile([128, TCH], F32, tag="mm", name=f"p1_{l}_{c}_{m}")
                for k in range(KD):
                    nc.tensor.matmul(pt[:], w1_t[:, k, ts(m, 128)],
                                     n2[:, k, cs],
                                     start=(k == 0), stop=(k == KD - 1))
                nc.scalar.activation(hT[:, m, :], pt[:], AF.Gelu,
                                     bias=b1_t[:, m:m + 1])
            return hT

        def add_f_lnnext(c, ar_f, b2_t, n_next):
            cs = ts(c, TCH)
            art = arr.tile([128, KD, TCH], BF16, tag="arr", name=f"artf{c}")
            nc.sync.dma_start(out=art[:], in_=ar_f[c][:])
            for m in range(KD):
                nc.vector.scalar_tensor_tensor(
                    out=xt[:, m, cs], in0=art[:, m, :],
                    scalar=b2_t[:, m:m + 1],
                    in1=xt[:, m, cs],
                    op0=ALU.add, op1=ALU.add)
            layer_norm_chunk(xt, n_next, c)

        for l in range(L):
            # ---------------- attention + Wo + AR, chunk-pipelined ----------------
            w1_t = load_w(_f("w1", l), KD, FFC, "w1", wff)
            w2_t = load_w(_f("w2", l), FFC // 128, D, "w2", wff)
            oT = op.tile([128, DQ // 128, T], BF16, tag="oT")
            n2 = npool.tile([128, KD, T], BF16, tag="n")
            ar_a = []

            def add_a(c):
                art = arr.tile([128, KD, TCH], BF16, tag="arr",
                               name=f"arta{c}_{l}")
                nc.sync.dma_start(out=art[:], in_=ar_a[c][:])
                for k in range(KD):
                    nc.vector.tensor_tensor(
                        out=xt[:, k, ts(c, TCH)],
                        in0=xt[:, k, ts(c, TCH)],
                        in1=art[:, k, :], op=ALU.add)

            def add_a_ln2_c0():
                add_a(0)
                layer_norm_chunk(xt, n2, 0)

            attn_chunk(l, 0, qT, kT, vhs2, oT)
            ar_a.append(mm_ar_chunk(wo_t, DQ // 128, oT, 0, False))
            attn_chunk(l, 1, qT, kT, vhs2, oT, inject=add_a_ln2_c0)
            ar_a.append(mm_ar_chunk(wo_t, DQ // 128, oT, 1, False))

            # ------- residual c1 + LN2(c1) overlap FFN(c0) -------
            b1_t = bias.tile([128, FFC // 128], F32, tag="bias")
            nc.sync.dma_start(out=b1_t[:], in_=inp[_f("b1", l)][:])
            b2_t = bias.tile([128, KD], F32, tag="bias")
            nc.sync.dma_start(out=b2_t[:], in_=inp[_f("b2", l)][:])
            add_a(1)
            layer_norm_chunk(xt, n2, 1)

            ar_f = []
            hT0 = ffn_w1(l, 0, n2, w1_t, b1_t)
            ar_f.append(mm_ar_chunk(w2_t, FFC // 128, hT0, 0, True))
            hT1 = ffn_w1(l, 1, n2, w1_t, b1_t)

            n_next = npool.tile([128, KD, T], BF16, tag="n")
            ar_f.append(mm_ar_chunk(w2_t, FFC // 128, hT1, 1, True))
            add_f_lnnext(0, ar_f, b2_t, n_next)

            if l < L - 1:
                bqkv_t = bias.tile([128, 6], F32, tag="bias")
                nc.sync.dma_start(out=bqkv_t[:], in_=inp[_f("bqkv", l + 1)][:])
                wq_t = load_w(_f("wq", l + 1), KD, DQ, "wq", wts)
                wk_t = load_w(_f("wk", l + 1), KD, DQ, "wk", wts)
                wv_t = load_w(_f("wv", l + 1), KD, DQ, "wv", wts)
                wo_t = load_w(_f("wo", l + 1), DQ // 128, D, "wo", wts)
                qT = qkv.tile([128, DQ // 128, T], BF16, tag="qT")
                kT = qkv.tile([128, DQ // 128, T], BF16, tag="kT")
                vT = qkv.tile([128, DQ // 128, T], BF16, tag="vT")
                vhs2 = make_vhs2(l + 1)
                qkv_head_chunk(0, n_next, qT, kT, vT, wq_t, wk_t, wv_t,
                               bqkv_t, vhs2)
            add_f_lnnext(1, ar_f, b2_t, n_next)
            if l < L - 1:
                qkv_head_chunk(1, n_next, qT, kT, vT, wq_t, wk_t, wv_t,
                               bqkv_t, vhs2)
            n_cur = n_next

        # after the loop, n_cur holds the final layer norm
        nf = n_cur
        attn_ctx.close()
        ps_lg = ctx.enter_context(
            tc.tile_pool(name="ps_lg", bufs=4, space="PSUM"))

        # logits^T [VSP, T]: emb strip is the stationary operand, nf moves.
        # A t2=0-only prefix runs first: it depends only on chunk 0 of the
        # final layernorm, hiding the last AllReduce + LN tail of chunk 1.
        NPRE = 28

        def logit_block(vb, t2, ebt):
            pt = ps_lg.tile([128, TCH], F32, tag="lg",
                            name=f"plg{vb}_{t2}")
            for k in range(KD):
                nc.tensor.matmul(pt[:], ebt[:, k, :],
                                 nf[:, k, ts(t2, TCH)],
                                 start=(k == 0), stop=(k == KD - 1))
            lo = lout.tile([128, TCH], BF16, tag="lo", name=f"lo{vb}_{t2}")
            nc.scalar.copy(lo[:], pt[:])
            nc.sync.dma_start(out=logits[ts(vb, 128), ts(t2, TCH)],
                              in_=lo[:])

        for vb in range(NPRE):
            ebt = embp.tile([128, KD, 128], BF16, tag="emb", name=f"ebA{vb}")
            nc.sync.dma_start(out=ebt[:], in_=inp["emb"][:, :, ts(vb, 128)])
            logit_block(vb, 0, ebt)
        for vb in range(VSP // 128):
            ebt = embp.tile([128, KD, 128], BF16, tag="emb", name=f"ebB{vb}")
            nc.sync.dma_start(out=ebt[:], in_=inp["emb"][:, :, ts(vb, 128)])
            for t2 in range(NTC):
                if t2 == 0 and vb < NPRE:
                    continue
                logit_block(vb, t2, ebt)


# ------------------------------------------------------------------
# Host side
# ------------------------------------------------------------------

def _bf(a):
    import ml_dtypes

    return np.asarray(a, np.float32).astype(ml_dtypes.bfloat16)


def _kfold(w):
    """[in, out] -> [128, in//128, out] K-tiled layout."""
    i, o = w.shape
    return np.ascontiguousarray(
        w.reshape(i // 128, 128, o).transpose(1, 0, 2))


def _cols(v):
    """[n] -> [128, n//128] per-partition bias columns."""
    return np.ascontiguousarray(v.reshape(-1, 128).T)


def prep_inputs(inputs):
    """Full inputs -> list of 8 per-core input maps."""
    f = lambda a: np.asarray(a, np.float32)
    tokens = np.asarray(inputs["tokens"])
    tok_emb, pos_emb = f(inputs["tok_emb"]), f(inputs["pos_emb"])
    ln1_g, ln1_b = f(inputs["ln1_g"]), f(inputs["ln1_b"])
    wq, wk = f(inputs["wq"]), f(inputs["wk"])
    wv, wo = f(inputs["wv"]), f(inputs["wo"])
    ln2_g, ln2_b = f(inputs["ln2_g"]), f(inputs["ln2_b"])
    w1, b1 = f(inputs["w1"]), f(inputs["b1"])
    w2, b2 = f(inputs["w2"]), f(inputs["b2"])
    lnf_g = f(inputs["lnf_g"])

    sc = 1.0 / np.sqrt(HD)
    x0 = tok_emb[tokens] + pos_emb[:S][None]          # [B, S, D]
    ones = np.ones((128, 1), np.float32)
    ident = np.eye(128, dtype=np.float32)
    # strictly-lower-triangular mask in [key, query] layout
    tri = np.tril(np.full((128, 128), MASK_VAL, np.float32), k=-1)

    in_maps = []
    for core in range(N_CORES):
        b = core // TP
        tpr = core % TP
        m = {
            "x0": _bf(_kfold(np.ascontiguousarray(x0[b].T))),
            "ones": _bf(ones), "ident": _bf(ident), "tri": _bf(tri),
        }
        qs = slice(tpr * DQ, (tpr + 1) * DQ)
        fs = slice(tpr * FFC, (tpr + 1) * FFC)
        for l in range(L):
            wql = wq[l][:, qs] * sc
            wkl = wk[l][:, qs]
            wvl = wv[l][:, qs]
            m[_f("wq", l)] = _bf(_kfold(ln1_g[l][:, None] * wql))
            m[_f("wk", l)] = _bf(_kfold(ln1_g[l][:, None] * wkl))
            m[_f("wv", l)] = _bf(_kfold(ln1_g[l][:, None] * wvl))
            m[_f("wo", l)] = _bf(_kfold(wo[l][qs, :]))
            m[_f("w1", l)] = _bf(_kfold(ln2_g[l][:, None] * w1[l][:, fs]))
            m[_f("w2", l)] = _bf(_kfold(w2[l][fs, :]))
            m[_f("bqkv", l)] = np.concatenate(
                [_cols(ln1_b[l] @ wql), _cols(ln1_b[l] @ wkl),
                 _cols(ln1_b[l] @ wvl)], axis=1).astype(np.float32)
            m[_f("b1", l)] = _cols(b1[l][fs] + ln2_b[l] @ w1[l][:, fs]).astype(
                np.float32)
            m[_f("b2", l)] = _cols(b2[l]).astype(np.float32)
        v0 = tpr * VS
        v1 = min(v0 + VS, V)
        epad = np.zeros((D, VSP), np.float32)
        epad[:, :v1 - v0] = (tok_emb[v0:v1] * lnf_g[None, :]).T
        m["emb"] = _bf(_kfold(epad))
        in_maps.append(m)
    return in_maps


_CACHED = {}


def _get_program():
    if "nc" not in _CACHED:
        _CACHED["nc"] = build_program()
    return _CACHED["nc"]


def run(inputs, trace=False, **kw):
    nc = _get_program()
    in_maps = prep_inputs(inputs)
    return run_bass_kernel_spmd(nc, in_maps, list(range(N_CORES)),
                                trace=trace, **kw)


def assemble(results, inputs):
    """Per-core logits -> full [B, S, V] float32."""
    lnf_b = np.asarray(inputs["lnf_b"], np.float32)
    tok_emb = np.asarray(inputs["tok_emb"], np.float32)
    out = np.empty((B, S, V), np.float32)
    for b in range(B):
        parts = []
        for tpr in range(TP):
            v0 = tpr * VS
            v1 = min(v0 + VS, V)
            parts.append(
                results[b * TP + tpr]["logits"][:v1 - v0, :].T.astype(
                    np.float32))
        out[b] = np.concatenate(parts, axis=1)
    if np.any(lnf_b):
        out += (tok_emb @ lnf_b)[None, None, :]
    return out


def kernel(**inputs):
    res = run(inputs)
    return assemble(res.results, inputs)


if __name__ == "__main__":
    print("building program...")
    build_program()
    print("build + compile OK")
